# revision 1
# baseline (speedup 1.0000x reference)
"""Trainium2 Bass kernel for the ExplicitV2C GNN layer (GATv2 message passing).

Strategy (8-core SPMD):
  * Host: permute nodes into 512 degree-balanced "bins" of 128 nodes; group
    edges by destination bin; pad each bin to S_SUB subtiles of 128 edges.
    Each core owns 64 bins (8192 nodes) and all edges targeting them.
  * Device per core:
      Phase 1 (nodes, sharded): LLR fusion (Linear+LN+ReLU+mask) computed in
        feature-major (transposed) layout; AllGather builds the full x_w table.
      Phase 2 (edges): per bin-window, gather x_w[src] rows via indirect DMA,
        compute GATv2 scores with leaky_relu decomposed as
        0.2*v + 0.8*relu(v) (linear part folded into att-projected weights),
        segment softmax + weighted aggregation via one-hot matmuls in PSUM.
      Phase 3 (nodes, sharded): degree gate + final LayerNorm.
  * Host: concatenate shards, undo the node permutation.
"""

import os
import sys

sys.path.insert(0, "/opt/trn_rl_repo")

import numpy as np
import ml_dtypes

import concourse.bass as bass
import concourse.bacc as bacc
import concourse.mybir as mybir
import concourse.tile as tile
from concourse.bass import IndirectOffsetOnAxis
from concourse.bass_utils import run_bass_kernel_spmd

F32 = mybir.dt.float32
BF16 = mybir.dt.bfloat16
I32 = mybir.dt.int32
AX = mybir.AxisListType
OP = mybir.AluOpType
AF = mybir.ActivationFunctionType

P = 128
NCORES = 8
LN_EPS = 1e-5
SM_EPS = 1e-16


class Cfg:
    def __init__(self, N=65536, E=262144, S_SUB=4, G=4):
        self.N, self.E, self.S_SUB, self.G = N, E, S_SUB, G
        self.BINS = N // P                       # node tiles (bins) total
        self.BPC = self.BINS // NCORES           # bins per core
        self.NSHARD = N // NCORES                # nodes per core
        assert self.BPC % G == 0
        self.NG = self.BPC // G                  # gather groups per core
        self.SLOTS = S_SUB * P                   # edge slots per bin
        self.TOTSUB = self.BPC * S_SUB           # subtiles per core
        self.EC = self.TOTSUB * P                # edge slots per core


# ----------------------------------------------------------------------------
# Host-side preprocessing
# ----------------------------------------------------------------------------

def _balance_bins(deg_in, N, BINS, target):
    """Assign each node to a (bin, slot); every bin gets exactly P nodes and
    edge load (sum of in-degrees) as close to `target` as possible."""
    order = np.argsort(-deg_in, kind="stable")
    bin_of = np.empty(N, np.int64)
    slot_of = np.empty(N, np.int64)
    bin_of[order] = np.arange(N) % BINS
    slot_of[order] = np.arange(N) // BINS
    loads = np.bincount(bin_of, weights=deg_in, minlength=BINS).astype(np.int64)

    # greedy repair: swap nodes between heaviest and lightest bins
    for _ in range(4000):
        a = int(np.argmax(loads))
        if loads[a] <= target:
            break
        b = int(np.argmin(loads))
        nodes_a = np.where(bin_of == a)[0]
        nodes_b = np.where(bin_of == b)[0]
        da = deg_in[nodes_a]
        db = deg_in[nodes_b]
        need = loads[a] - target
        # choose u in a, v in b maximizing min(du-dv, need) without overshooting b
        best = None
        du_max = da.max()
        for u_idx in np.argsort(-da)[:8]:
            u = nodes_a[u_idx]
            du = deg_in[u]
            want = du - need  # ideal partner degree
            v_idx = int(np.argmin(np.abs(db - max(want, 0))))
            v = nodes_b[v_idx]
            dv = deg_in[v]
            if du <= dv:
                continue
            gain = du - dv
            if loads[b] + gain > target + du_max:
                continue
            best = (u, v, gain)
            break
        if best is None:
            break
        u, v, gain = best
        bu, su = bin_of[u], slot_of[u]
        bin_of[u], slot_of[u] = bin_of[v], slot_of[v]
        bin_of[v], slot_of[v] = bu, su
        loads[a] -= gain
        loads[b] += gain
    return bin_of, slot_of, loads


def host_prep(cfg, inputs):
    N, E, G = cfg.N, cfg.E, cfg.G
    BINS, BPC, NSHARD = cfg.BINS, cfg.BPC, cfg.NSHARD

    x = np.asarray(inputs["x"], np.float32)
    ei = np.asarray(inputs["edge_index"])
    src_o = ei[0].astype(np.int64)
    dst_o = ei[1].astype(np.int64)
    ea = np.asarray(inputs["edge_attr"], np.float32)
    ndeg = np.asarray(inputs["node_degrees"]).astype(np.int64)
    llr = np.asarray(inputs["llr_features"], np.float32).reshape(N)
    vmask = np.asarray(inputs["var_node_mask"]).astype(np.float32).reshape(N)

    deg_in = np.bincount(dst_o, minlength=N).astype(np.int64)
    target = -(-E // BINS)  # ceil
    bin_of, slot_of, loads = _balance_bins(deg_in, N, BINS, target)
    max_load = int(loads.max())
    S_SUB = max(1, -(-max_load // P))
    cfg = Cfg(N, E, S_SUB, G)
    SLOTS = cfg.SLOTS

    o2p = bin_of * P + slot_of
    p2o = np.argsort(o2p)          # p2o[pid] = original id

    # --- edge arrays grouped by destination bin --------------------------------
    src_p = o2p[src_o]
    dst_pid = o2p[dst_o]
    ebin = dst_pid >> 7
    eslot = dst_pid & 127

    eorder = np.argsort(ebin, kind="stable")
    ebin_s = ebin[eorder]
    starts = np.zeros(BINS + 1, np.int64)
    np.cumsum(np.bincount(ebin_s, minlength=BINS), out=starts[1:])
    rank = np.arange(E) - starts[ebin_s]
    q = ebin_s * SLOTS + rank                 # position in padded layout

    esrc = np.zeros(BINS * SLOTS, np.int32)
    eslot_f = np.full(BINS * SLOTS, float(P), np.float32)   # pad slot = P (dead)
    eattr = np.zeros((BINS * SLOTS, 8), np.float32)
    esrc[q] = src_p[eorder]
    eslot_f[q] = eslot[eorder].astype(np.float32)
    eattr[q] = ea[eorder]

    S = S_SUB
    esrc_r = esrc.reshape(NCORES, BPC, S, P)
    eslot_r = eslot_f.reshape(NCORES, BPC, S, P)
    eattr_r = eattr.reshape(NCORES, BPC, S, P, 8)

    # gather indices: [core, group, p, w*S+j] flattened to [NG, P*G*S]
    idx_g = (
        esrc_r.reshape(NCORES, cfg.NG, G, S, P)
        .transpose(0, 1, 4, 2, 3)
        .reshape(NCORES, cfg.NG, P * G * S)
        .astype(np.int32)
    )
    # dst slots: [core, p, bin*S+j]
    dst_col = (
        eslot_r.transpose(0, 3, 1, 2).reshape(NCORES, P, BPC * S).astype(np.float32)
    )
    # edge attrs transposed: [core, 8, (bin*S+j)*P + p]
    ea_t = (
        eattr_r.transpose(0, 4, 1, 2, 3).reshape(NCORES, 8, BPC * S * P)
    ).astype(np.float32)

    # --- node arrays -----------------------------------------------------------
    xp = x[p2o]                                          # [N,HID] permuted
    x_t = xp.reshape(NCORES, NSHARD, P).transpose(0, 2, 1).copy()  # [c,128,NSHARD]
    lm = np.stack(
        [llr[p2o].reshape(NCORES, NSHARD), vmask[p2o].reshape(NCORES, NSHARD)], axis=1
    ).astype(np.float32)                                 # [c, 2, NSHARD]

    degc = np.clip(ndeg, 0, 99)[p2o].reshape(NCORES, BPC, P)
    deg_arr = degc.transpose(0, 2, 1).reshape(NCORES, P * BPC).astype(np.int32)

    # --- weights ---------------------------------------------------------------
    w = {k: np.asarray(v, np.float32) for k, v in inputs.items()
         if k not in ("x", "edge_index", "edge_attr", "node_degrees",
                      "llr_features", "var_node_mask")}
    att = w["att"]                                       # [4,128]
    W_l, W_r, W_e = w["W_l"], w["W_r"], w["W_e"]
    consts = {
        "c_Wfx": w["W_f"][:P].copy(),
        "c_wfl": w["W_f"][P:P + 1].copy(),
        "c_bfb": np.broadcast_to(w["b_f"].reshape(1, P), (P, P)).copy(),
        "c_gfb": np.broadcast_to(w["g_f"].reshape(1, P), (P, P)).copy(),
        "c_befb": np.broadcast_to(w["be_f"].reshape(1, P), (P, P)).copy(),
        "c_Wl": W_l.copy(),
        "c_Wr": W_r.copy(),
        "c_We": W_e.copy(),
        "c_wla": 0.2 * (W_l.reshape(P, 4, P) * att[None]).sum(-1),
        "c_wra": 0.2 * (W_r.reshape(P, 4, P) * att[None]).sum(-1),
        "c_wea": 0.2 * (W_e.reshape(8, 4, P) * att[None]).sum(-1),
        "c_attb": np.broadcast_to(
            att.reshape(1, 512), (P, 512)).astype(ml_dtypes.bfloat16).copy(),
        "c_iota": np.broadcast_to(
            np.arange(P, dtype=np.float32)[None, :], (P, P)).copy(),
        "c_ident": np.eye(P, dtype=np.float32),
        "c_Wg1x": w["W_g1"][:P].copy(),
        "c_Wg2": w["W_g2"].copy(),
        "c_gg": w["g_g"].reshape(P, 1).copy(),
        "c_beg": w["be_g"].reshape(P, 1).copy(),
        "c_gob": np.broadcast_to(w["g_o"].reshape(1, P), (P, P)).copy(),
        "c_bob": np.broadcast_to(w["b_o"].reshape(1, P), (P, P)).copy(),
        "c_bg2b": np.broadcast_to(w["b_g2"].reshape(1, P), (P, P)).copy(),
        "c_Td": (w["deg_emb"] @ w["W_g1"][P:P + 16] + w["b_g1"][None, :]).copy(),
    }
    consts = {k: np.ascontiguousarray(v) for k, v in consts.items()}

    x_rows = xp.reshape(NCORES, NSHARD, P)
    mask_col = (
        vmask[p2o].reshape(NCORES, BPC, P).transpose(0, 2, 1).copy()
    )  # [c, 128, BPC]

    in_maps = []
    for k in range(NCORES):
        m = {
            "x_t": x_t[k], "lm": lm[k], "idx_g": idx_g[k],
            "dst_col": dst_col[k], "ea_t": ea_t[k], "deg_arr": deg_arr[k],
            "x_rows": np.ascontiguousarray(x_rows[k]),
            "mask_col": np.ascontiguousarray(mask_col[k]),
        }
        m.update(consts)
        in_maps.append(m)
    return cfg, in_maps, p2o


# ----------------------------------------------------------------------------
# Device kernel
# ----------------------------------------------------------------------------

def build_kernel(cfg):
    N, S, G = cfg.N, cfg.S_SUB, cfg.G
    BPC, NSHARD, NG = cfg.BPC, cfg.NSHARD, cfg.NG
    PHASES = int(os.environ.get("GNN_PHASES", "3"))

    nc = bacc.Bacc("TRN2", target_bir_lowering=False, debug=False,
                   num_devices=NCORES)

    # ---- I/O ----
    d_xt = nc.dram_tensor("x_t", [P, NSHARD], F32, kind="ExternalInput")
    d_lm = nc.dram_tensor("lm", [2, NSHARD], F32, kind="ExternalInput")
    d_idx = nc.dram_tensor("idx_g", [NG, P * G * S], I32, kind="ExternalInput")
    d_dst = nc.dram_tensor("dst_col", [P, BPC * S], F32, kind="ExternalInput")
    d_eat = nc.dram_tensor("ea_t", [8, BPC * S * P], F32, kind="ExternalInput")
    d_deg = nc.dram_tensor("deg_arr", [P * BPC], I32, kind="ExternalInput")
    d_xrows = nc.dram_tensor("x_rows", [NSHARD, P], F32, kind="ExternalInput")
    d_mcol = nc.dram_tensor("mask_col", [P, BPC], F32, kind="ExternalInput")
    d_out = nc.dram_tensor("y", [NSHARD, P], F32, kind="ExternalOutput")

    cshape = {
        "c_Wfx": ([P, P], F32), "c_wfl": ([1, P], F32), "c_bfb": ([P, P], F32),
        "c_gfb": ([P, P], F32), "c_befb": ([P, P], F32),
        "c_Wl": ([P, 512], F32), "c_Wr": ([P, 512], F32), "c_We": ([8, 512], F32),
        "c_wla": ([P, 4], F32), "c_wra": ([P, 4], F32), "c_wea": ([8, 4], F32),
        "c_attb": ([P, 512], BF16), "c_iota": ([P, P], F32),
        "c_ident": ([P, P], F32),
        "c_Wg1x": ([P, P], F32), "c_Wg2": ([P, P], F32),
        "c_gg": ([P, 1], F32), "c_beg": ([P, 1], F32),
        "c_gob": ([P, P], F32), "c_bob": ([P, P], F32), "c_bg2b": ([P, P], F32),
        "c_Td": ([100, P], F32),
    }
    d_c = {k: nc.dram_tensor(k, sh, dt, kind="ExternalInput")
           for k, (sh, dt) in cshape.items()}

    # internal DRAM for the AllGather'd x_w table
    d_xw_shard = nc.dram_tensor("xw_shard", [NSHARD, P], F32)
    d_xw_full = nc.dram_tensor("xw_full", [N, P], F32, addr_space="Shared")
    d_td = nc.dram_tensor("td_tab", [100, P], F32)

    with tile.TileContext(nc) as tc:
        with (
            tc.tile_pool(name="const", bufs=1) as cpool,
            tc.tile_pool(name="resid", bufs=1) as rpool,
        ):
            C = {}
            for k, (sh, dt) in cshape.items():
                C[k] = cpool.tile(sh, dt, tag=k, name=f"const_{k}")
                nc.sync.dma_start(out=C[k][:], in_=d_c[k].ap())
            # T_d table -> internal DRAM (gather source)
            nc.sync.dma_start(out=d_td.ap(), in_=d_c["c_Td"].ap())

            xt_s = rpool.tile([P, NSHARD], F32, tag="xt")       # x^T shard
            lm_s = rpool.tile([2, NSHARD], F32, tag="lm")
            xwt_s = rpool.tile([P, NSHARD], F32, tag="xwt")     # x_w^T shard
            v2c_s = rpool.tile([P, NSHARD], F32, tag="v2c")     # aggregated out
            nc.sync.dma_start(out=xt_s[:], in_=d_xt.ap())
            nc.sync.dma_start(out=lm_s[:], in_=d_lm.ap())

            NT = BPC  # node tiles per core

            # ================= Phase 1: LLR fusion ==========================
            with (
                tc.tile_pool(name="p1psum", bufs=2, space="PSUM") as pp1,
                tc.tile_pool(name="p1tr", bufs=2, space="PSUM") as pp1t,
                tc.tile_pool(name="p1sb", bufs=3) as sb1,
                tc.tile_pool(name="p1in", bufs=1) as sb1c,
            ):
                mcol = sb1c.tile([P, NT], F32, tag="mcol")
                nc.sync.dma_start(out=mcol[:], in_=d_mcol.ap())
                for t in range(NT):
                    ns = slice(t * P, (t + 1) * P)
                    py = pp1.tile([P, P], F32, tag="y")
                    nc.tensor.matmul(py[:], C["c_Wfx"][:], xt_s[:, ns],
                                     start=True, stop=False)
                    nc.tensor.matmul(py[:], C["c_wfl"][:], lm_s[0:1, ns],
                                     start=False, stop=True)
                    ytT = sb1.tile([P, P], F32, tag="ytT")
                    nc.scalar.activation(ytT[:], py[:], AF.Identity)
                    ptr = pp1t.tile([P, P], F32, tag="tr")
                    nc.tensor.transpose(ptr[:], ytT[:], C["c_ident"][:])
                    yr = sb1.tile([P, P], F32, tag="yr")  # [node, feat] + b_f
                    nc.vector.tensor_tensor(out=yr[:], in0=ptr[:],
                                            in1=C["c_bfb"][:], op=OP.add)
                    xrw = sb1.tile([P, P], F32, tag="xrw")
                    nc.sync.dma_start(out=xrw[:], in_=d_xrows.ap()[ns, :])
                    # LayerNorm over features (free axis)
                    s2 = sb1.tile([P, 1], F32, tag="s2")
                    sqs = sb1.tile([P, P], F32, tag="sqs")
                    nc.scalar.activation(sqs[:], yr[:], AF.Square,
                                         accum_out=s2[:])
                    s1 = sb1.tile([P, 1], F32, tag="s1")
                    nc.vector.reduce_sum(out=s1[:], in_=yr[:], axis=AX.X)
                    mu = sb1.tile([P, 1], F32, tag="mu")
                    nc.vector.tensor_scalar(out=mu[:], in0=s1[:],
                                            scalar1=1.0 / P, scalar2=None,
                                            op0=OP.mult)
                    mu2 = sb1.tile([P, 1], F32, tag="mu2")
                    nc.vector.tensor_tensor(out=mu2[:], in0=mu[:], in1=mu[:],
                                            op=OP.mult)
                    var = sb1.tile([P, 1], F32, tag="var")
                    nc.vector.scalar_tensor_tensor(
                        out=var[:], in0=s2[:], scalar=1.0 / P, in1=mu2[:],
                        op0=OP.mult, op1=OP.subtract)
                    nc.vector.tensor_scalar(out=var[:], in0=var[:],
                                            scalar1=LN_EPS, scalar2=None,
                                            op0=OP.add)
                    sd = sb1.tile([P, 1], F32, tag="sd")
                    nc.scalar.activation(sd[:], var[:], AF.Sqrt)
                    ivs = sb1.tile([P, 1], F32, tag="ivs")
                    nc.vector.reciprocal(ivs[:], sd[:])
                    nmi = sb1.tile([P, 1], F32, tag="nmi")
                    nc.vector.scalar_tensor_tensor(
                        out=nmi[:], in0=mu[:], scalar=-1.0, in1=ivs[:],
                        op0=OP.mult, op1=OP.mult)
                    zn = sb1.tile([P, P], F32, tag="zn")
                    nc.scalar.activation(zn[:], yr[:], AF.Identity,
                                         scale=ivs[:], bias=nmi[:])
                    nc.vector.tensor_tensor(out=zn[:], in0=zn[:],
                                            in1=C["c_gfb"][:], op=OP.mult)
                    nc.vector.tensor_tensor(out=zn[:], in0=zn[:],
                                            in1=C["c_befb"][:], op=OP.add)
                    fu = sb1.tile([P, P], F32, tag="fu")
                    nc.scalar.activation(fu[:], zn[:], AF.Relu)
                    # x_w = x + mask*(fused - x)   (mask per node = partition)
                    d1 = sb1.tile([P, P], F32, tag="d1")
                    nc.vector.tensor_tensor(out=d1[:], in0=fu[:], in1=xrw[:],
                                            op=OP.subtract)
                    xwr = sb1.tile([P, P], F32, tag="xwr")
                    nc.vector.scalar_tensor_tensor(
                        out=xwr[:], in0=d1[:], scalar=mcol[:, t:t + 1],
                        in1=xrw[:], op0=OP.mult, op1=OP.add)
                    nc.sync.dma_start(out=d_xw_shard.ap()[ns, :], in_=xwr[:])
                    # transposed copy for the xr-side matmuls
                    ptw = pp1t.tile([P, P], F32, tag="tr")
                    nc.tensor.transpose(ptw[:], xwr[:], C["c_ident"][:])
                    nc.scalar.activation(xwt_s[:, ns], ptw[:], AF.Identity)

            if PHASES >= 2:
                # AllGather the x_w table
                nc.gpsimd.collective_compute(
                    "AllGather", OP.bypass,
                    replica_groups=[list(range(NCORES))],
                    ins=[d_xw_shard.ap().opt()],
                    outs=[d_xw_full.ap().opt()],
                )
            else:
                for t in range(NT):
                    ns = slice(t * P, (t + 1) * P)
                    nc.sync.dma_start(out=d_out.ap()[ns, :],
                                      in_=xwt_s[:, ns])

            # ================= Phase 2: edges ===============================
            with (
                tc.tile_pool(name="pz", bufs=2, space="PSUM") as ppz,
                tc.tile_pool(name="pxl", bufs=2, space="PSUM") as ppxl,
                tc.tile_pool(name="po4", bufs=1, space="PSUM") as ppo4,
                tc.tile_pool(name="psm", bufs=2, space="PSUM") as ppsm,
                tc.tile_pool(name="ptr", bufs=1, space="PSUM") as pptr,
                tc.tile_pool(name="e_in", bufs=2) as ein,
                tc.tile_pool(name="e_wk", bufs=3) as ewk,
                tc.tile_pool(name="e_wk2", bufs=2) as ewk2,
            ):
                for g in range(NG if PHASES >= 2 else 0):
                    idx_t = ein.tile([P, G * S], I32, tag="idx")
                    nc.sync.dma_start(
                        out=idx_t[:],
                        in_=d_idx.ap()[g, :].rearrange("(p j) -> p j", p=P))
                    xg_t = ein.tile([P, G * S, P], F32, tag="xg")
                    for jj in range(G * S):
                        nc.gpsimd.indirect_dma_start(
                            out=xg_t[:, jj, :], out_offset=None,
                            in_=d_xw_full.ap(),
                            in_offset=IndirectOffsetOnAxis(
                                ap=idx_t[:, jj:jj + 1], axis=0),
                        )
                    dst_t = ein.tile([P, G * S], F32, tag="dst")
                    nc.sync.dma_start(
                        out=dst_t[:],
                        in_=d_dst.ap()[:, g * G * S:(g + 1) * G * S])
                    eat_t = ein.tile([8, G * S * P], F32, tag="eat")
                    nc.sync.dma_start(
                        out=eat_t[:],
                        in_=d_eat.ap()[:, g * G * S * P:(g + 1) * G * S * P])

                    for wdx in range(G):
                        win = g * G + wdx
                        ns = slice(win * P, (win + 1) * P)
                        # xr tile for this window's nodes
                        pxr = ppz.tile([P, 512], F32, tag="z")
                        nc.tensor.matmul(pxr[:], xwt_s[:, ns], C["c_Wr"][:],
                                         start=True, stop=True)
                        xr_sb = ewk2.tile([P, 512], F32, tag="xr")
                        nc.scalar.activation(xr_sb[:], pxr[:], AF.Identity)
                        pxra = ppsm.tile([P, 4], F32, tag="sm")
                        nc.tensor.matmul(pxra[:], xwt_s[:, ns], C["c_wra"][:],
                                         start=True, stop=True)
                        xra_sb = ewk2.tile([P, 4], F32, tag="xra")
                        nc.vector.tensor_copy(out=xra_sb[:], in_=pxra[:])

                        pden = ppsm.tile([P, 4], F32, tag="sm")
                        po4 = ppo4.tile([P, 512], F32, tag="o4")

                        for j in range(S):
                            st = wdx * S + j
                            xg = xg_t[:, st, :]
                            ptp = pptr.tile([P, P], F32, tag="tr")
                            nc.tensor.transpose(ptp[:], xg, C["c_ident"][:])
                            xgT = ewk.tile([P, P], F32, tag="xgT")
                            nc.scalar.activation(xgT[:], ptp[:], AF.Identity)

                            S_sb = ewk.tile([P, P], F32, tag="S")
                            nc.vector.tensor_tensor(
                                out=S_sb[:],
                                in0=dst_t[:, st:st + 1].to_broadcast([P, P]),
                                in1=C["c_iota"][:], op=OP.is_equal)
                            pts = pptr.tile([P, P], F32, tag="tr")
                            nc.tensor.transpose(pts[:], S_sb[:], C["c_ident"][:])
                            st_sb = ewk.tile([P, P], F32, tag="st")
                            nc.scalar.activation(st_sb[:], pts[:], AF.Identity)

                            ea_sl = eat_t[:, st * P:(st + 1) * P]
                            # z = xl + xr[dst] + ea@We  (xl also into its own bank)
                            pxl = ppxl.tile([P, 512], F32, tag="xl")
                            pal = ppsm.tile([P, 4], F32, tag="sm")
                            pz = ppz.tile([P, 512], F32, tag="z")
                            nc.tensor.matmul(pz[:], xgT[:], C["c_Wl"][:],
                                             start=True, stop=False)
                            nc.tensor.matmul(pxl[:], xgT[:], C["c_Wl"][:],
                                             start=True, stop=True)
                            nc.tensor.matmul(pal[:], xgT[:], C["c_wla"][:],
                                             start=True, stop=False)
                            nc.tensor.matmul(pz[:], st_sb[:], xr_sb[:],
                                             start=False, stop=False)
                            nc.tensor.matmul(pal[:], st_sb[:], xra_sb[:],
                                             start=False, stop=False)
                            nc.tensor.matmul(pz[:], ea_sl, C["c_We"][:],
                                             start=False, stop=True)
                            nc.tensor.matmul(pal[:], ea_sl, C["c_wea"][:],
                                             start=False, stop=True)

                            r_sb = ewk.tile([P, 512], BF16, tag="r")
                            nc.scalar.activation(r_sb[:], pz[:], AF.Relu)
                            zat = ewk.tile([P, 512], BF16, tag="zat")
                            nc.vector.tensor_tensor(out=zat[:], in0=r_sb[:],
                                                    in1=C["c_attb"][:],
                                                    op=OP.mult)
                            alr = ewk.tile([P, 4], F32, tag="alr")
                            nc.vector.reduce_sum(
                                out=alr[:],
                                in_=zat[:].rearrange("p (h c) -> p h c", h=4),
                                axis=AX.X)
                            alpha = ewk.tile([P, 4], F32, tag="alpha")
                            # alpha = 0.8*relu_part + lin_part
                            nc.vector.scalar_tensor_tensor(
                                out=alpha[:], in0=alr[:], scalar=0.8,
                                in1=pal[:], op0=OP.mult, op1=OP.add)
                            au = ewk.tile([P, 4], F32, tag="au")
                            nc.scalar.activation(au[:], alpha[:], AF.Exp)
                            nc.tensor.matmul(pden[:], S_sb[:], au[:],
                                             start=(j == 0), stop=(j == S - 1))
                            # xl scaled by per-edge attention (per head)
                            xla = ewk.tile([P, 512], F32, tag="xla")
                            for h in range(4):
                                hs = slice(h * P, (h + 1) * P)
                                nc.vector.tensor_scalar(
                                    out=xla[:, hs], in0=pxl[:, hs],
                                    scalar1=au[:, h:h + 1], scalar2=None,
                                    op0=OP.mult)
                            for h in range(4):
                                hs = slice(h * P, (h + 1) * P)
                                nc.tensor.matmul(po4[:, hs], S_sb[:],
                                                 xla[:, hs],
                                                 start=(j == 0 and h == 0),
                                                 stop=(j == S - 1 and h == 3))
                        # normalize + head mean -> v2c
                        dv = ewk.tile([P, 4], F32, tag="dv")
                        nc.vector.tensor_scalar(out=dv[:], in0=pden[:],
                                                scalar1=SM_EPS, scalar2=None,
                                                op0=OP.add)
                        iv = ewk.tile([P, 4], F32, tag="iv")
                        nc.vector.reciprocal(iv[:], dv[:])
                        nc.vector.tensor_scalar(out=iv[:], in0=iv[:],
                                                scalar1=0.25, scalar2=None,
                                                op0=OP.mult)
                        vsl = v2c_s[:, ns]
                        nc.vector.tensor_scalar(
                            out=vsl, in0=po4[:, 0:P], scalar1=iv[:, 0:1],
                            scalar2=None, op0=OP.mult)
                        for h in range(1, 4):
                            hs = slice(h * P, (h + 1) * P)
                            nc.vector.scalar_tensor_tensor(
                                out=vsl, in0=po4[:, hs], scalar=iv[:, h:h + 1],
                                in1=vsl, op0=OP.mult, op1=OP.add)

            if PHASES == 2:
                for t in range(NT):
                    ns = slice(t * P, (t + 1) * P)
                    nc.sync.dma_start(out=d_out.ap()[ns, :], in_=v2c_s[:, ns])

            # ================= Phase 3: degree gate + final LN ==============
            with (
                tc.tile_pool(name="p3a", bufs=2, space="PSUM") as pp3,
                tc.tile_pool(name="p3t", bufs=2, space="PSUM") as pp3t,
                tc.tile_pool(name="g_in", bufs=1) as gin,
                tc.tile_pool(name="g_wk", bufs=3) as gwk,
            ):
              if PHASES >= 3:
                degi = gin.tile([P, BPC], I32, tag="degi")
                nc.sync.dma_start(
                    out=degi[:],
                    in_=d_deg.ap().rearrange("(p t) -> p t", p=P))
                dterm = gin.tile([P, BPC, P], F32, tag="dterm")
                for t in range(BPC):
                    nc.gpsimd.indirect_dma_start(
                        out=dterm[:, t, :], out_offset=None,
                        in_=d_td.ap(),
                        in_offset=IndirectOffsetOnAxis(
                            ap=degi[:, t:t + 1], axis=0),
                    )
                for t in range(NT):
                    ns = slice(t * P, (t + 1) * P)
                    ptv = pp3t.tile([P, P], F32, tag="t")
                    nc.tensor.transpose(ptv[:], v2c_s[:, ns], C["c_ident"][:])
                    v2cT = gwk.tile([P, P], F32, tag="v2cT")
                    nc.scalar.activation(v2cT[:], ptv[:], AF.Identity)
                    ph = pp3.tile([P, P], F32, tag="h")
                    nc.tensor.matmul(ph[:], v2cT[:], C["c_Wg1x"][:],
                                     start=True, stop=True)
                    h_sb = gwk.tile([P, P], F32, tag="h_sb")
                    nc.vector.tensor_tensor(out=h_sb[:], in0=ph[:],
                                            in1=dterm[:, t, :], op=OP.add)
                    # LN stats (free axis = features)
                    s2 = gwk.tile([P, 1], F32, tag="s2")
                    sqs = gwk.tile([P, P], F32, tag="sqs")
                    nc.scalar.activation(sqs[:], h_sb[:], AF.Square,
                                         accum_out=s2[:])
                    s1 = gwk.tile([P, 1], F32, tag="s1")
                    nc.vector.reduce_sum(out=s1[:], in_=h_sb[:], axis=AX.X)
                    mu = gwk.tile([P, 1], F32, tag="mu")
                    nc.vector.tensor_scalar(out=mu[:], in0=s1[:], scalar1=1.0 / P,
                                            scalar2=None, op0=OP.mult)
                    mu2 = gwk.tile([P, 1], F32, tag="mu2")
                    nc.vector.tensor_tensor(out=mu2[:], in0=mu[:], in1=mu[:],
                                            op=OP.mult)
                    var = gwk.tile([P, 1], F32, tag="var")
                    nc.vector.scalar_tensor_tensor(
                        out=var[:], in0=s2[:], scalar=1.0 / P, in1=mu2[:],
                        op0=OP.mult, op1=OP.subtract)
                    nc.vector.tensor_scalar(out=var[:], in0=var[:],
                                            scalar1=LN_EPS, scalar2=None,
                                            op0=OP.add)
                    sd = gwk.tile([P, 1], F32, tag="sd")
                    nc.scalar.activation(sd[:], var[:], AF.Sqrt)
                    ivs = gwk.tile([P, 1], F32, tag="ivs")
                    nc.vector.reciprocal(ivs[:], sd[:])
                    nmi = gwk.tile([P, 1], F32, tag="nmi")
                    nc.vector.scalar_tensor_tensor(
                        out=nmi[:], in0=mu[:], scalar=-1.0, in1=ivs[:],
                        op0=OP.mult, op1=OP.mult)
                    zn = gwk.tile([P, P], F32, tag="zn")
                    nc.scalar.activation(zn[:], h_sb[:], AF.Identity,
                                         scale=ivs[:], bias=nmi[:])
                    ptz = pp3t.tile([P, P], F32, tag="t")
                    nc.tensor.transpose(ptz[:], zn[:], C["c_ident"][:])
                    h2T = gwk.tile([P, P], F32, tag="h2T")
                    nc.scalar.activation(h2T[:], ptz[:], AF.Relu,
                                         scale=C["c_gg"][:], bias=C["c_beg"][:])
                    pg = pp3.tile([P, P], F32, tag="h")
                    nc.tensor.matmul(pg[:], h2T[:], C["c_Wg2"][:],
                                     start=True, stop=True)
                    gpre = gwk.tile([P, P], F32, tag="gpre")
                    nc.vector.tensor_tensor(out=gpre[:], in0=pg[:],
                                            in1=C["c_bg2b"][:], op=OP.add)
                    gate = gwk.tile([P, P], F32, tag="gate")
                    nc.scalar.activation(gate[:], gpre[:], AF.Sigmoid)
                    p_sb = gwk.tile([P, P], F32, tag="p_sb")
                    nc.vector.tensor_tensor(out=p_sb[:], in0=v2c_s[:, ns],
                                            in1=gate[:], op=OP.mult)
                    # final LN
                    fs2 = gwk.tile([P, 1], F32, tag="fs2")
                    fsq = gwk.tile([P, P], F32, tag="fsq")
                    nc.scalar.activation(fsq[:], p_sb[:], AF.Square,
                                         accum_out=fs2[:])
                    fs1 = gwk.tile([P, 1], F32, tag="fs1")
                    nc.vector.reduce_sum(out=fs1[:], in_=p_sb[:], axis=AX.X)
                    fmu = gwk.tile([P, 1], F32, tag="fmu")
                    nc.vector.tensor_scalar(out=fmu[:], in0=fs1[:],
                                            scalar1=1.0 / P, scalar2=None,
                                            op0=OP.mult)
                    fmu2 = gwk.tile([P, 1], F32, tag="fmu2")
                    nc.vector.tensor_tensor(out=fmu2[:], in0=fmu[:], in1=fmu[:],
                                            op=OP.mult)
                    fvar = gwk.tile([P, 1], F32, tag="fvar")
                    nc.vector.scalar_tensor_tensor(
                        out=fvar[:], in0=fs2[:], scalar=1.0 / P, in1=fmu2[:],
                        op0=OP.mult, op1=OP.subtract)
                    nc.vector.tensor_scalar(out=fvar[:], in0=fvar[:],
                                            scalar1=LN_EPS, scalar2=None,
                                            op0=OP.add)
                    fsd = gwk.tile([P, 1], F32, tag="fsd")
                    nc.scalar.activation(fsd[:], fvar[:], AF.Sqrt)
                    fiv = gwk.tile([P, 1], F32, tag="fiv")
                    nc.vector.reciprocal(fiv[:], fsd[:])
                    fnmi = gwk.tile([P, 1], F32, tag="fnmi")
                    nc.vector.scalar_tensor_tensor(
                        out=fnmi[:], in0=fmu[:], scalar=-1.0, in1=fiv[:],
                        op0=OP.mult, op1=OP.mult)
                    zf = gwk.tile([P, P], F32, tag="zf")
                    nc.scalar.activation(zf[:], p_sb[:], AF.Identity,
                                         scale=fiv[:], bias=fnmi[:])
                    y1 = gwk.tile([P, P], F32, tag="y1")
                    nc.vector.tensor_tensor(out=y1[:], in0=zf[:],
                                            in1=C["c_gob"][:], op=OP.mult)
                    y2 = gwk.tile([P, P], F32, tag="y2")
                    nc.vector.tensor_tensor(out=y2[:], in0=y1[:],
                                            in1=C["c_bob"][:], op=OP.add)
                    nc.sync.dma_start(out=d_out.ap()[ns, :], in_=y2[:])

    nc.compile()
    return nc


# ----------------------------------------------------------------------------
# Entry point
# ----------------------------------------------------------------------------

_CACHE = {}


def _get_kernel(cfg):
    key = (cfg.N, cfg.E, cfg.S_SUB, cfg.G)
    if key not in _CACHE:
        _CACHE[key] = build_kernel(cfg)
    return _CACHE[key]


def bench_hw(nc, in_maps, iters=32):
    """Build the sharded PJRT callable once; time repeated executions.

    Returns (per_core_results, per_iter_ns). Mirrors the tail of
    bass2jax.run_bass_via_pjrt so the jit cache is reused across calls.
    """
    import time
    import jax
    from jax.sharding import Mesh, PartitionSpec
    from jax.experimental.shard_map import shard_map
    import concourse.mybir as mb
    from concourse import bass2jax as b2j

    b2j.install_neuronx_cc_hook()
    n_cores = len(in_maps)
    partition_name = (nc.partition_id_tensor.name
                      if nc.partition_id_tensor else None)
    in_names, out_names, out_avals, zero_outs = [], [], [], []
    for alloc in nc.m.functions[0].allocations:
        if not isinstance(alloc, mb.MemoryLocationSet):
            continue
        name = alloc.memorylocations[0].name
        if alloc.kind == "ExternalInput":
            if name != partition_name:
                in_names.append(name)
        elif alloc.kind == "ExternalOutput":
            out_names.append(name)
            shape = tuple(alloc.tensor_shape)
            dtype = mb.dt.np(alloc.dtype)
            out_avals.append(jax.core.ShapedArray(shape, dtype))
            zero_outs.append(np.zeros(shape, dtype))
    n_params = len(in_names)
    n_outs = len(out_avals)
    in_names.extend(out_names)
    if partition_name is not None:
        in_names.append(partition_name)
    donate = tuple(range(n_params, n_params + n_outs))

    def _body(*args):
        operands = list(args)
        if partition_name is not None:
            operands.append(b2j.partition_id_tensor())
        outs = b2j._bass_exec_p.bind(
            *operands,
            out_avals=tuple(out_avals), in_names=tuple(in_names),
            out_names=tuple(out_names), lowering_input_output_aliases=(),
            sim_require_finite=True, sim_require_nnan=True, nc=nc)
        return tuple(outs)

    devices = jax.devices()[:n_cores]
    mesh = Mesh(np.asarray(devices), ("core",))
    sharded = jax.jit(
        shard_map(_body, mesh=mesh,
                  in_specs=(PartitionSpec("core"),) * (n_params + n_outs),
                  out_specs=(PartitionSpec("core"),) * n_outs,
                  check_rep=False),
        donate_argnums=donate, keep_unused=True)

    concat_in = [
        np.concatenate([np.asarray(in_maps[c][in_names[i]])
                        for c in range(n_cores)], axis=0)
        for i in range(n_params)]
    from jax.sharding import NamedSharding
    in_shardings = [NamedSharding(mesh, PartitionSpec("core"))] * n_params
    in_bufs = [jax.device_put(a, s) for a, s in zip(concat_in, in_shardings)]

    def fresh_zeros():
        return [jax.device_put(
            np.zeros((n_cores * z.shape[0], *z.shape[1:]), z.dtype),
            NamedSharding(mesh, PartitionSpec("core"))) for z in zero_outs]

    # warmup + correctness output
    out_arrs = sharded(*in_bufs, *fresh_zeros())
    jax.block_until_ready(out_arrs)
    results = [
        {name: np.asarray(out_arrs[i]).reshape(n_cores, *out_avals[i].shape)[c]
         for i, name in enumerate(out_names)}
        for c in range(n_cores)]

    zsets = [fresh_zeros() for _ in range(iters)]
    t0 = time.perf_counter()
    outs = [sharded(*in_bufs, *z) for z in zsets]
    jax.block_until_ready(outs)
    dt = (time.perf_counter() - t0) / iters
    return results, dt * 1e9


def kernel(**inputs):
    global LAST_EXEC_NS
    N, E = 65536, 262144
    cfg = Cfg(N, E)
    cfg, in_maps, p2o = host_prep(cfg, inputs)
    nc = _get_kernel(cfg)
    if bool(int(os.environ.get("GNN_BENCH", "0"))):
        results, ns = bench_hw(nc, in_maps,
                               iters=int(os.environ.get("GNN_ITERS", "32")))
        LAST_EXEC_NS = ns
    else:
        res = run_bass_kernel_spmd(nc, in_maps, core_ids=list(range(NCORES)))
        results = res.results
        LAST_EXEC_NS = res.exec_time_ns
    y_perm = np.concatenate([results[k]["y"] for k in range(NCORES)], axis=0)
    y = np.empty_like(y_perm)
    y[p2o] = y_perm
    return y.astype(np.float32)


LAST_EXEC_NS = None



# revision 5
# speedup vs baseline: 48.4067x; 48.4067x over previous
"""Trainium2 Bass kernel for the ExplicitV2C GNN layer (GATv2 message passing).

Strategy (8-core SPMD, no collectives):
  * Host: permute nodes into 512 degree-balanced bins of 128 nodes; group
    edges by destination bin; pad each bin to S subtiles of 128 edges.
    Each core owns 64 bins (8192 dst nodes) and all edges targeting them.
  * Device per core:
      Phase 1 (replicated): LLR fusion (Linear+LN+ReLU+mask) over ALL nodes;
        writes the full bf16 x_w table to core-local DRAM (gather source).
      Phase 2 (edges, sharded): batched indirect-DMA gathers of x_w rows
        (2560 rows per DMA op, including each window's own dst nodes), GATv2
        scores with bf16 matmuls, leaky_relu as 0.2*z + 0.8*relu(z),
        segment softmax + weighted aggregation via one-hot matmuls in PSUM.
      Phase 3 (nodes, sharded): degree gate + final LayerNorm; the degree
        embedding term is fetched with a single dma_gather op.
  * Host: reorder the output shards, undo the node permutation.
"""

import os
import sys

sys.path.insert(0, "/opt/trn_rl_repo")

import numpy as np
import ml_dtypes

import concourse.bass as bass
import concourse.bacc as bacc
import concourse.mybir as mybir
import concourse.tile as tile
from concourse.bass import IndirectOffsetOnAxis
from concourse.bass_utils import run_bass_kernel_spmd

F32 = mybir.dt.float32
BF16 = mybir.dt.bfloat16
I32 = mybir.dt.int32
I16 = mybir.dt.int16
AX = mybir.AxisListType
OP = mybir.AluOpType
AF = mybir.ActivationFunctionType

P = 128
NCORES = 8
LN_EPS = 1e-5
SM_EPS = 1e-16
BF = ml_dtypes.bfloat16


class Cfg:
    def __init__(self, N=65536, E=262144, S_SUB=4):
        self.N, self.E, self.S_SUB = N, E, S_SUB
        self.BINS = N // P                       # node bins total (512)
        self.BPC = self.BINS // NCORES           # windows per core (64)
        self.NSHARD = N // NCORES                # nodes per core (8192)
        self.SLOTS = S_SUB * P                   # edge slots per bin
        self.NSLAB = N // 512                    # phase-1 slabs (128)
        self.CH = 16                             # gather chunks per core
        self.WPC = self.BPC // self.CH           # windows per chunk (4)
        self.CPW = S_SUB + 1                     # gather cols per window


# ----------------------------------------------------------------------------
# Host-side preprocessing
# ----------------------------------------------------------------------------

def _balance_bins(deg_in, N, BINS, target):
    """LPT assignment: nodes by in-degree descending onto the lightest bin
    that still has free slots; every bin gets exactly P nodes."""
    import heapq
    order = np.argsort(-deg_in, kind="stable")
    bin_of = np.empty(N, np.int64)
    slot_of = np.empty(N, np.int64)
    heap = [(0, 0, b) for b in range(BINS)]
    heapq.heapify(heap)
    for n in order:
        while True:
            load, cnt, b = heapq.heappop(heap)
            if cnt < P:
                break
        bin_of[n] = b
        slot_of[n] = cnt
        heapq.heappush(heap, (load + int(deg_in[n]), cnt + 1, b))
    loads = np.bincount(bin_of, weights=deg_in, minlength=BINS).astype(np.int64)
    return bin_of, slot_of, loads


def host_prep(cfg, inputs):
    N, E = cfg.N, cfg.E
    BINS, BPC, NSHARD = cfg.BINS, cfg.BPC, cfg.NSHARD

    x = np.asarray(inputs["x"], np.float32)
    ei = np.asarray(inputs["edge_index"])
    src_o = ei[0].astype(np.int64)
    dst_o = ei[1].astype(np.int64)
    ea = np.asarray(inputs["edge_attr"], np.float32)
    ndeg = np.asarray(inputs["node_degrees"]).astype(np.int64)
    llr = np.asarray(inputs["llr_features"], np.float32).reshape(N)
    vmask = np.asarray(inputs["var_node_mask"]).astype(np.float32).reshape(N)

    deg_in = np.bincount(dst_o, minlength=N).astype(np.int64)
    target = -(-E // BINS)
    bin_of, slot_of, loads = _balance_bins(deg_in, N, BINS, target)
    max_load = int(loads.max())
    S = max(1, -(-max_load // P))
    cfg = Cfg(N, E, S)
    SLOTS = cfg.SLOTS
    CH, WPC, CPW = cfg.CH, cfg.WPC, cfg.CPW

    # permuted node id: node o sits at (bin, slot)
    o2p = bin_of * P + slot_of
    p2o = np.argsort(o2p)          # p2o[pid] = original id

    # x_w DRAM table row of permuted node n: n = slab*512 + t*128 + p is
    # stored at row slab*512 + p*4 + t (matches contiguous slab stores)
    n_ids = np.arange(N)
    n_slab = n_ids // 512
    n_t = (n_ids % 512) // P
    n_p = n_ids % P
    row_of_node = n_slab * 512 + n_p * 4 + n_t

    # --- edge arrays grouped by destination bin ---------------------------
    src_p = o2p[src_o]
    dst_pid = o2p[dst_o]
    ebin = dst_pid >> 7
    eslot = dst_pid & 127

    eorder = np.argsort(ebin, kind="stable")
    ebin_s = ebin[eorder]
    starts = np.zeros(BINS + 1, np.int64)
    np.cumsum(np.bincount(ebin_s, minlength=BINS), out=starts[1:])
    rank = np.arange(E) - starts[ebin_s]
    q = ebin_s * SLOTS + rank                 # position in padded layout

    esrc = np.zeros(BINS * SLOTS, np.int64)   # permuted src node id
    eslot_f = np.full(BINS * SLOTS, float(P), np.float32)   # pad slot = P
    eattr = np.zeros((BINS * SLOTS, 8), np.float32)
    esrc[q] = src_p[eorder]
    eslot_f[q] = eslot[eorder].astype(np.float32)
    eattr[q] = ea[eorder]

    # gather row index per edge slot (into the shuffled x_w table layout).
    # Pad slots point at the last table row (positive int16 after re-basing),
    # and each bin's slots are stably partitioned so that positive-row slots
    # come last: the transpose-gather drops trailing NEGATIVE indices, so the
    # final index of every per-window gather op must be non-negative.
    egrow_f = np.full(BINS * SLOTS, N - 1, np.int64)
    filled = np.zeros(BINS * SLOTS, bool)
    filled[q] = True
    egrow_f[q] = row_of_node[src_p[eorder]]
    eg2 = egrow_f.reshape(BINS, SLOTS)
    es2 = eslot_f.reshape(BINS, SLOTS)
    ea2 = eattr.reshape(BINS, SLOTS, 8)
    order2 = np.argsort(eg2 >= N // 2, axis=1, kind="stable")
    eg2 = np.take_along_axis(eg2, order2, axis=1)
    es2 = np.take_along_axis(es2, order2, axis=1)
    ea2 = np.take_along_axis(ea2, order2[:, :, None], axis=1)
    eslot_f = es2.reshape(-1)
    eattr = ea2.reshape(-1, 8)
    egrow = eg2.reshape(BINS, S, P)                   # [win_glob, j, p]

    # per-core transpose-gather indices: int16 = table_row - N/2 (sign trick
    # extends the addressable range to 65536 rows).  Position i = col*128 + e;
    # the CPW cols of window w are [own nodes, edge subtile 0..S-1].
    NPC = WPC * CPW * P                           # idx positions per chunk
    idx_g = np.zeros((NCORES, CH, P, NPC // 16), np.int16)
    half = N // 2
    for c in range(NCORES):
        for ch in range(CH):
            unw = np.zeros(NPC, np.int64)
            for wdx in range(WPC):
                wg = c * BPC + ch * WPC + wdx     # global bin
                base = wdx * CPW * P
                own_nodes = wg * P + np.arange(P) # permuted ids of own bin
                unw[base:base + P] = row_of_node[own_nodes]
                for j in range(S):
                    unw[base + (1 + j) * P:base + (2 + j) * P] = egrow[wg, j]
            w16 = (unw - half).astype(np.int16).reshape(NPC // 16, 16).T
            idx_g[c, ch] = np.tile(w16, (8, 1))

    eslot_r = eslot_f.reshape(NCORES, BPC * S, P)
    dst_col = eslot_r.transpose(0, 2, 1).copy()               # [c, p, col]
    eattr_r = eattr.reshape(NCORES, BPC * S, P, 8)
    ea_t = eattr_r.transpose(0, 3, 1, 2).reshape(
        NCORES, 8, BPC * S * P).astype(BF)                    # [c, 8, col*p]

    # --- node arrays (full, replicated) -----------------------------------
    xp = x[p2o]                                              # [N, HID]
    x_t_full = np.ascontiguousarray(xp.T.astype(BF))         # [128, N]
    # interleaved rows: [slab, p, t, f], node n = slab*512 + t*128 + p
    xr4 = np.ascontiguousarray(
        xp.reshape(cfg.NSLAB, 4, P, P).transpose(0, 2, 1, 3).astype(BF))
    # llr per node: [p, slab, t]
    llr4 = np.ascontiguousarray(
        llr[p2o].reshape(cfg.NSLAB, 4, P).transpose(2, 0, 1).astype(BF))
    # mask: [p, slab, t]
    m_all = np.ascontiguousarray(
        vmask[p2o].reshape(cfg.NSLAB, 4, P).transpose(2, 0, 1).astype(BF))

    # degree gather indices (int16), wrap order, replicated to 128 parts
    degc = np.clip(ndeg, 0, 99)[p2o].reshape(NCORES, NSHARD).astype(np.int16)
    deg_wrap = degc.reshape(NCORES, NSHARD // 16, 16).transpose(0, 2, 1)
    deg_rep = np.ascontiguousarray(np.tile(deg_wrap, (1, 8, 1)))  # [c,128,S]

    # --- weights -----------------------------------------------------------
    w = {k: np.asarray(v, np.float32) for k, v in inputs.items()
         if k not in ("x", "edge_index", "edge_attr", "node_degrees",
                      "llr_features", "var_node_mask")}
    att = w["att"]                                           # [4,128]

    def bcast_row(v, reps):                                  # [P, reps*128]
        return np.ascontiguousarray(
            np.broadcast_to(np.tile(v, reps)[None, :], (P, reps * P)))

    flags = {
        "gf1": bool(np.allclose(w["g_f"], 1.0)),
        "bef0": bool(np.allclose(w["be_f"], 0.0)),
        "gg1": bool(np.allclose(w["g_g"], 1.0)),
        "beg0": bool(np.allclose(w["be_g"], 0.0)),
        "bg20": bool(np.allclose(w["b_g2"], 0.0)),
        "go1": bool(np.allclose(w["g_o"], 1.0)),
        "bo0": bool(np.allclose(w["b_o"], 0.0)),
    }

    consts = {
        "c_Wfx": w["W_f"][:P].astype(BF),                    # [128,128]
        "c_wfl4": np.ascontiguousarray(np.broadcast_to(
            np.tile(w["W_f"][P], 4)[None, :], (P, 512)).astype(BF)),
        "c_bfc": np.ascontiguousarray(
            w["b_f"].reshape(P, 1).astype(np.float32)),
        "c_eps": np.full((P, 1), LN_EPS, np.float32),
        "c_Wl": w["W_l"].astype(BF),                         # [128,512]
        "c_Wr": w["W_r"].astype(BF),
        "c_We": w["W_e"].astype(BF),                         # [8,512]
        "c_attb": np.ascontiguousarray(
            np.broadcast_to(att.reshape(1, 512), (P, 512)).astype(BF)),
        "c_iota": np.ascontiguousarray(
            np.broadcast_to(np.arange(P, dtype=np.float32)[None, :], (P, P))),
        "c_ident": np.eye(P, dtype=BF),
        "c_Wg1x": w["W_g1"][:P].astype(BF),                  # [128,128]
        "c_Wg2": w["W_g2"].astype(BF),
        "c_Td": (w["deg_emb"] @ w["W_g1"][P:P + 16]
                 + w["b_g1"][None, :]).astype(np.float32),   # [100,128]
    }
    if not flags["gf1"]:
        consts["c_gf4"] = bcast_row(w["g_f"], 4).astype(BF)
    if not flags["bef0"]:
        consts["c_bef4"] = bcast_row(w["be_f"], 4).astype(BF)
    if not flags["gg1"]:
        consts["c_gg4"] = bcast_row(w["g_g"], 4).astype(BF)
    if not flags["beg0"]:
        consts["c_beg4"] = bcast_row(w["be_g"], 4).astype(BF)
    if not flags["bg20"]:
        consts["c_bg24"] = bcast_row(w["b_g2"], 4).astype(np.float32)
    if not flags["go1"]:
        consts["c_go4"] = bcast_row(w["g_o"], 4).astype(np.float32)
    if not flags["bo0"]:
        consts["c_bo4"] = bcast_row(w["b_o"], 4).astype(np.float32)
    consts = {k: np.ascontiguousarray(v) for k, v in consts.items()}

    in_maps = []
    for k in range(NCORES):
        m = {
            "x_t": x_t_full, "xr4": xr4, "llr4": llr4, "m_all": m_all,
            "idx_g": np.ascontiguousarray(idx_g[k]),
            "dst_col": np.ascontiguousarray(dst_col[k]),
            "ea_t": np.ascontiguousarray(ea_t[k]),
            "deg_w": deg_rep[k],
        }
        m.update(consts)
        in_maps.append(m)
    return cfg, in_maps, p2o, flags


# ----------------------------------------------------------------------------
# Device kernel
# ----------------------------------------------------------------------------

def build_kernel(cfg, flags):
    PH = int(os.environ.get("GNN_PH", "3"))
    N, S = cfg.N, cfg.S_SUB
    BPC, NSHARD = cfg.BPC, cfg.NSHARD
    NSLAB, CH, WPC, CPW = cfg.NSLAB, cfg.CH, cfg.WPC, cfg.CPW
    SH_SLAB = NSHARD // 512                      # output slabs per core (16)
    COLS = BPC * S                               # edge subtile columns (256)

    nc = bacc.Bacc("TRN2", target_bir_lowering=False, debug=False,
                   num_devices=NCORES)

    # ---- I/O ----
    d_xt = nc.dram_tensor("x_t", [P, N], BF16, kind="ExternalInput")
    d_xr4 = nc.dram_tensor("xr4", [NSLAB, P, 4, P], BF16, kind="ExternalInput")
    d_llr = nc.dram_tensor("llr4", [P, NSLAB, 4], BF16, kind="ExternalInput")
    d_m = nc.dram_tensor("m_all", [P, NSLAB, 4], BF16, kind="ExternalInput")
    d_idx = nc.dram_tensor("idx_g", [CH, P, WPC * CPW * 8], I16,
                           kind="ExternalInput")
    d_dst = nc.dram_tensor("dst_col", [P, COLS], F32, kind="ExternalInput")
    d_eat = nc.dram_tensor("ea_t", [8, COLS * P], BF16, kind="ExternalInput")
    d_deg = nc.dram_tensor("deg_w", [P, NSHARD // 16], I16,
                           kind="ExternalInput")
    d_out = nc.dram_tensor("y", [SH_SLAB, P, 4, P], F32, kind="ExternalOutput")

    cshape = {
        "c_Wfx": ([P, P], BF16), "c_wfl4": ([P, 512], BF16),
        "c_bfc": ([P, 1], F32), "c_eps": ([P, 1], F32),
        "c_Wl": ([P, 512], BF16), "c_Wr": ([P, 512], BF16),
        "c_We": ([8, 512], BF16), "c_attb": ([P, 512], BF16),
        "c_iota": ([P, P], F32), "c_ident": ([P, P], BF16),
        "c_Wg1x": ([P, P], BF16), "c_Wg2": ([P, P], BF16),
        "c_Td": ([100, P], F32),
    }
    for nm, fl, dt in (("c_gf4", "gf1", BF16), ("c_bef4", "bef0", BF16),
                       ("c_gg4", "gg1", BF16), ("c_beg4", "beg0", BF16),
                       ("c_bg24", "bg20", F32), ("c_go4", "go1", F32),
                       ("c_bo4", "bo0", F32)):
        if not flags[fl]:
            cshape[nm] = ([P, 512], dt)
    d_c = {k: nc.dram_tensor(k, sh, dt, kind="ExternalInput")
           for k, (sh, dt) in cshape.items()}

    # gather table with a shadow copy of the first half appended: the
    # transpose-gather reads rows [0,N) via int16 idx relative to base N/2,
    # but its declared AP is rows [N/2, 2N) -- the shadow writes make every
    # phase-1 store overlap that range so the dep tracker orders them.
    d_xw = nc.dram_tensor("xw_tab", [2 * N, P], BF16)
    d_td = nc.dram_tensor("td_tab", [100, P], F32)

    with tile.TileContext(nc) as tc:
        with (
            tc.tile_pool(name="const", bufs=1) as cpool,
            tc.tile_pool(name="resid", bufs=1) as rpool,
        ):
            C = {}
            for k, (sh, dt) in cshape.items():
                C[k] = cpool.tile(sh, dt, tag=k, name=f"const_{k}")
                nc.sync.dma_start(out=C[k][:], in_=d_c[k].ap())
            nc.sync.dma_start(out=d_td.ap(), in_=C["c_Td"][:])

            llr_s = rpool.tile([P, NSLAB, 4], BF16, tag="llr")
            nc.sync.dma_start(out=llr_s[:], in_=d_llr.ap())
            m_s = rpool.tile([P, NSLAB, 4], BF16, tag="m")
            nc.sync.dma_start(out=m_s[:], in_=d_m.ap())
            dst_t = rpool.tile([P, COLS], F32, tag="dst")
            nc.sync.dma_start(out=dst_t[:], in_=d_dst.ap())
            eat_t = rpool.tile([8, COLS * P], BF16, tag="eat")
            nc.sync.dma_start(out=eat_t[:], in_=d_eat.ap())
            deg_t = rpool.tile([P, NSHARD // 16], I16, tag="deg")
            nc.sync.dma_start(out=deg_t[:], in_=d_deg.ap())

            v2c_nm = rpool.tile([P, BPC, P], BF16, tag="v2c")  # [p, w, f]
            dterm = rpool.tile([P, BPC, P], F32, tag="dterm")

            # degree-embedding term for all own nodes (1024-idx chunks --
            # a single instruction's descriptors must fit the SWDGE ring)
            for g in range(NSHARD // 1024):
                nc.gpsimd.dma_gather(
                    out_ap=dterm[:, g * 8:(g + 1) * 8, :], in_ap=d_td.ap(),
                    idxs_ap=deg_t[:, g * 64:(g + 1) * 64],
                    num_idxs=1024, num_idxs_reg=1024, elem_size=P,
                    transpose=False)

            # ================= Phase 1: LLR fusion (replicated) =============
            with (
                tc.tile_pool(name="p1mm", bufs=2, space="PSUM") as pp1,
                tc.tile_pool(name="p1tr", bufs=2, space="PSUM") as pp1t,
                tc.tile_pool(name="p1in", bufs=3) as sb1i,
                tc.tile_pool(name="p1wk", bufs=2) as sb1,
            ):
                for s in range(NSLAB):
                    ns = slice(s * 512, (s + 1) * 512)
                    xt_sl = sb1i.tile([P, 512], BF16, tag="xt")
                    nc.sync.dma_start(out=xt_sl[:], in_=d_xt.ap()[:, ns])
                    py = pp1.tile([P, 512], F32, tag="y")
                    nc.tensor.matmul(py[:], C["c_Wfx"][:], xt_sl[:],
                                     start=True, stop=True)
                    ytT = sb1.tile([P, 512], BF16, tag="ytT")
                    nc.scalar.activation(ytT[:], py[:], AF.Identity,
                                         bias=C["c_bfc"][:])
                    ptq = pp1t.tile([P, 512], BF16, tag="tr")
                    for t in range(4):
                        qs = slice(t * P, (t + 1) * P)
                        nc.tensor.transpose(ptq[:, qs], ytT[:, qs],
                                            C["c_ident"][:])
                    wl4 = sb1.tile([P, 4, P], BF16, tag="wl4")
                    nc.vector.tensor_tensor(
                        out=wl4[:],
                        in0=C["c_wfl4"][:].rearrange("p (t f) -> p t f", t=4),
                        in1=llr_s[:, s, :].rearrange("p (t o) -> p t o", o=1)
                            .to_broadcast([P, 4, P]), op=OP.mult)
                    yr = sb1.tile([P, 4, P], BF16, tag="yr")
                    nc.vector.tensor_tensor(
                        out=yr[:], in0=ptq[:].rearrange("p (t f) -> p t f", t=4),
                        in1=wl4[:], op=OP.add)
                    bst = sb1.tile([P, 4, 6], F32, tag="bst")
                    mv = sb1.tile([P, 4, 2], F32, tag="mv")
                    for t in range(4):
                        nc.vector.bn_stats(bst[:, t, :], yr[:, t, :])
                        nc.vector.bn_aggr(mv[:, t, :], bst[:, t, :])
                    sd4 = sb1.tile([P, 4], F32, tag="sd4")
                    nc.scalar.activation(sd4[:], mv[:, :, 1], AF.Sqrt,
                                         bias=C["c_eps"][:])
                    iv4 = sb1.tile([P, 4], F32, tag="iv4")
                    nc.vector.reciprocal(iv4[:], sd4[:])
                    nm4 = sb1.tile([P, 4], F32, tag="nm4")
                    nc.vector.scalar_tensor_tensor(
                        out=nm4[:], in0=mv[:, :, 0], scalar=-1.0, in1=iv4[:],
                        op0=OP.mult, op1=OP.mult)
                    t1 = sb1.tile([P, 4, P], BF16, tag="t1")
                    nc.vector.tensor_tensor(
                        out=t1[:], in0=yr[:],
                        in1=iv4[:].rearrange("p (t o) -> p t o", o=1)
                            .to_broadcast([P, 4, P]), op=OP.mult)
                    t2 = sb1.tile([P, 4, P], BF16, tag="t2")
                    nc.vector.tensor_tensor(
                        out=t2[:], in0=t1[:],
                        in1=nm4[:].rearrange("p (t o) -> p t o", o=1)
                            .to_broadcast([P, 4, P]), op=OP.add)
                    zz = t2
                    if not flags["gf1"]:
                        zg = sb1.tile([P, 4, P], BF16, tag="zg")
                        nc.vector.tensor_tensor(
                            out=zg[:], in0=zz[:],
                            in1=C["c_gf4"][:].rearrange(
                                "p (t f) -> p t f", t=4), op=OP.mult)
                        zz = zg
                    if not flags["bef0"]:
                        zb = sb1.tile([P, 4, P], BF16, tag="zb")
                        nc.vector.tensor_tensor(
                            out=zb[:], in0=zz[:],
                            in1=C["c_bef4"][:].rearrange(
                                "p (t f) -> p t f", t=4), op=OP.add)
                        zz = zb
                    fu = sb1.tile([P, 4, P], BF16, tag="fu")
                    nc.scalar.activation(fu[:], zz[:], AF.Relu)
                    xr_sl = sb1i.tile([P, 4, P], BF16, tag="xr")
                    nc.scalar.dma_start(out=xr_sl[:], in_=d_xr4.ap()[s])
                    d1 = sb1.tile([P, 4, P], BF16, tag="d1")
                    nc.vector.tensor_tensor(out=d1[:], in0=fu[:], in1=xr_sl[:],
                                            op=OP.subtract)
                    dm = sb1.tile([P, 4, P], BF16, tag="dm")
                    nc.vector.tensor_tensor(
                        out=dm[:], in0=d1[:],
                        in1=m_s[:, s, :].rearrange("p (t o) -> p t o", o=1)
                            .to_broadcast([P, 4, P]), op=OP.mult)
                    xw_sl = sb1.tile([P, 4, P], BF16, tag="xw")
                    nc.vector.tensor_tensor(out=xw_sl[:], in0=dm[:],
                                            in1=xr_sl[:], op=OP.add)
                    nc.sync.dma_start(
                        out=d_xw.ap()[ns, :].rearrange(
                            "(pp t) f -> pp t f", t=4),
                        in_=xw_sl[:])
                    if s < NSLAB // 2:
                        sh = slice(N + s * 512, N + (s + 1) * 512)
                        nc.scalar.dma_start(
                            out=d_xw.ap()[sh, :].rearrange(
                                "(pp t) f -> pp t f", t=4),
                            in_=xw_sl[:])
                    if PH == 1 and s < SH_SLAB:
                        dbg = sb1.tile([P, 4, P], F32, tag="dbg")
                        nc.vector.tensor_copy(out=dbg[:], in_=xw_sl[:])
                        nc.scalar.dma_start(out=d_out.ap()[s], in_=dbg[:])

            # ================= Phase 2: edges ===============================
            with (
                tc.tile_pool(name="pz", bufs=2, space="PSUM") as ppz,
                tc.tile_pool(name="pxl", bufs=2, space="PSUM") as ppxl,
                tc.tile_pool(name="po4", bufs=1, space="PSUM") as ppo4,
                tc.tile_pool(name="psm", bufs=1, space="PSUM") as ppsm,
                tc.tile_pool(name="ptr", bufs=2, space="PSUM") as pptr,
                tc.tile_pool(name="e_in", bufs=2) as ein,
                tc.tile_pool(name="e_wk", bufs=3) as ewk,
                tc.tile_pool(name="e_w2", bufs=2) as ewk2,
            ):
                NPC = WPC * CPW * P
                NPW = CPW * P                      # idx positions per window
                for ch in range(CH if PH >= 2 else 0):
                    idx_t = ein.tile([P, NPC // 16], I16, tag="idx")
                    nc.sync.dma_start(out=idx_t[:], in_=d_idx.ap()[ch])
                    # feature-major gather: column i holds x_w of idx i
                    xg_t = ein.tile([P, 1, NPC], BF16, tag="xg")
                    for wdx in range(WPC):
                        nc.gpsimd.dma_gather(
                            out_ap=xg_t[:, :, wdx * NPW:(wdx + 1) * NPW],
                            in_ap=d_xw.ap()[N // 2:2 * N, :],
                            idxs_ap=idx_t[:, wdx * (NPW // 16):
                                          (wdx + 1) * (NPW // 16)],
                            num_idxs=NPW, num_idxs_reg=NPW,
                            elem_size=P, transpose=True)

                    for wdx in range(WPC):
                        win = ch * WPC + wdx
                        base = wdx * CPW * P
                        # xr tile for this window's own (dst) nodes
                        xwT = xg_t[:, 0, base:base + P]
                        pxr = ppz.tile([P, 512], F32, tag="z")
                        nc.tensor.matmul(pxr[:], xwT, C["c_Wr"][:],
                                         start=True, stop=True)
                        xr_sb = ewk2.tile([P, 512], BF16, tag="xr")
                        nc.vector.tensor_copy(out=xr_sb[:], in_=pxr[:])

                        pden = ppsm.tile([P, 4], F32, tag="sm")
                        po4 = ppo4.tile([P, 512], F32, tag="o4")

                        for j in range(S):
                            st = win * S + j
                            # one-hot S [e, d] and its transpose
                            S_sb = ewk.tile([P, P], BF16, tag="S")
                            nc.vector.tensor_tensor(
                                out=S_sb[:],
                                in0=dst_t[:, st:st + 1].to_broadcast([P, P]),
                                in1=C["c_iota"][:], op=OP.is_equal)
                            pts = pptr.tile([P, P], BF16, tag="tr")
                            nc.tensor.transpose(pts[:], S_sb[:],
                                                C["c_ident"][:])
                            st_sb = ewk.tile([P, P], BF16, tag="st")
                            nc.scalar.activation(st_sb[:], pts[:], AF.Identity)
                            # gathered x_w[src] columns (feature-major)
                            xgT = xg_t[:, 0, base + (1 + j) * P:
                                       base + (2 + j) * P]

                            ea_sl = eat_t[:, st * P:(st + 1) * P]
                            pz = ppz.tile([P, 512], F32, tag="z")
                            pxl = ppxl.tile([P, 512], F32, tag="xl")
                            nc.tensor.matmul(pz[:], xgT, C["c_Wl"][:],
                                             start=True, stop=False)
                            nc.tensor.matmul(pxl[:], xgT, C["c_Wl"][:],
                                             start=True, stop=True)
                            nc.tensor.matmul(pz[:], st_sb[:], xr_sb[:],
                                             start=False, stop=False)
                            nc.tensor.matmul(pz[:], ea_sl, C["c_We"][:],
                                             start=False, stop=True)

                            # leaky = 0.2*z + 0.8*relu(z)
                            r_sb = ewk.tile([P, 512], BF16, tag="r")
                            nc.scalar.activation(r_sb[:], pz[:], AF.Relu,
                                                 scale=0.8)
                            lk = ewk.tile([P, 512], BF16, tag="lk")
                            nc.vector.scalar_tensor_tensor(
                                out=lk[:], in0=pz[:], scalar=0.2, in1=r_sb[:],
                                op0=OP.mult, op1=OP.add)
                            # alpha[e,h] = sum_c lk*att
                            zat = ewk.tile([P, 512], BF16, tag="zat")
                            nc.vector.tensor_tensor(out=zat[:], in0=lk[:],
                                                    in1=C["c_attb"][:],
                                                    op=OP.mult)
                            alpha = ewk.tile([P, 4], F32, tag="alpha")
                            nc.vector.reduce_sum(
                                out=alpha[:],
                                in_=zat[:].rearrange("p (h c) -> p h c", h=4),
                                axis=AX.X)
                            au = ewk.tile([P, 4], BF16, tag="au")
                            nc.scalar.activation(au[:], alpha[:], AF.Exp)
                            nc.tensor.matmul(pden[:], S_sb[:], au[:],
                                             start=(j == 0), stop=(j == S - 1))
                            # xl scaled by per-edge attention (per head)
                            xla = ewk.tile([P, 4, P], BF16, tag="xla")
                            nc.vector.tensor_tensor(
                                out=xla[:],
                                in0=pxl[:].rearrange("p (h f) -> p h f", h=4),
                                in1=au[:].rearrange("p (h o) -> p h o", o=1)
                                    .to_broadcast([P, 4, P]), op=OP.mult)
                            nc.tensor.matmul(
                                po4[:], S_sb[:],
                                xla[:].rearrange("p h f -> p (h f)"),
                                start=(j == 0), stop=(j == S - 1))
                        # normalize + head mean -> v2c (node-major)
                        dv = ewk.tile([P, 4], F32, tag="dv")
                        nc.vector.tensor_scalar(out=dv[:], in0=pden[:],
                                                scalar1=SM_EPS, scalar2=None,
                                                op0=OP.add)
                        iv = ewk.tile([P, 4], F32, tag="iv")
                        nc.vector.reciprocal(iv[:], dv[:])
                        nc.vector.tensor_scalar(out=iv[:], in0=iv[:],
                                                scalar1=0.25, scalar2=None,
                                                op0=OP.mult)
                        vsl = v2c_nm[:, win, :]
                        nc.vector.tensor_scalar(
                            out=vsl, in0=po4[:, 0:P], scalar1=iv[:, 0:1],
                            scalar2=None, op0=OP.mult)
                        for h in range(1, 4):
                            hs = slice(h * P, (h + 1) * P)
                            nc.vector.scalar_tensor_tensor(
                                out=vsl, in0=po4[:, hs], scalar=iv[:, h:h + 1],
                                in1=vsl, op0=OP.mult, op1=OP.add)

            # ================= Phase 3: degree gate + final LN ==============
            with (
                tc.tile_pool(name="p3a", bufs=2, space="PSUM") as pp3,
                tc.tile_pool(name="p3t", bufs=2, space="PSUM") as pp3t,
                tc.tile_pool(name="g_wk", bufs=2) as gwk,
            ):
                for sl in range(SH_SLAB if PH >= 3 else 0):
                    ws = slice(sl * 4, sl * 4 + 4)
                    # h = v2c @ Wg1x + dterm (deg-emb term incl. b_g1)
                    ph = pp3.tile([P, 512], F32, tag="h")
                    for t in range(4):
                        win = sl * 4 + t
                        ptv = pp3t.tile([P, P], BF16, tag="t")
                        nc.tensor.transpose(ptv[:], v2c_nm[:, win, :],
                                            C["c_ident"][:])
                        v2cT = gwk.tile([P, P], BF16, tag="v2cT")
                        nc.scalar.activation(v2cT[:], ptv[:], AF.Identity)
                        nc.tensor.matmul(ph[:, t * P:(t + 1) * P], v2cT[:],
                                         C["c_Wg1x"][:], start=True, stop=True)
                    h_sb = gwk.tile([P, 4, P], BF16, tag="h_sb")
                    nc.vector.tensor_tensor(
                        out=h_sb[:],
                        in0=ph[:].rearrange("p (t f) -> p t f", t=4),
                        in1=dterm[:, ws, :], op=OP.add)
                    bst = gwk.tile([P, 4, 6], F32, tag="bst")
                    mv = gwk.tile([P, 4, 2], F32, tag="mv")
                    for t in range(4):
                        nc.vector.bn_stats(bst[:, t, :], h_sb[:, t, :])
                        nc.vector.bn_aggr(mv[:, t, :], bst[:, t, :])
                    sd4 = gwk.tile([P, 4], F32, tag="sd4")
                    nc.scalar.activation(sd4[:], mv[:, :, 1], AF.Sqrt,
                                         bias=C["c_eps"][:])
                    iv4 = gwk.tile([P, 4], F32, tag="iv4")
                    nc.vector.reciprocal(iv4[:], sd4[:])
                    nm4 = gwk.tile([P, 4], F32, tag="nm4")
                    nc.vector.scalar_tensor_tensor(
                        out=nm4[:], in0=mv[:, :, 0], scalar=-1.0, in1=iv4[:],
                        op0=OP.mult, op1=OP.mult)
                    t1 = gwk.tile([P, 4, P], BF16, tag="t1")
                    nc.vector.tensor_tensor(
                        out=t1[:], in0=h_sb[:],
                        in1=iv4[:].rearrange("p (t o) -> p t o", o=1)
                            .to_broadcast([P, 4, P]), op=OP.mult)
                    t2 = gwk.tile([P, 4, P], BF16, tag="t2")
                    nc.vector.tensor_tensor(
                        out=t2[:], in0=t1[:],
                        in1=nm4[:].rearrange("p (t o) -> p t o", o=1)
                            .to_broadcast([P, 4, P]), op=OP.add)
                    zz = t2
                    if not flags["gg1"]:
                        zg = gwk.tile([P, 4, P], BF16, tag="zg")
                        nc.vector.tensor_tensor(
                            out=zg[:], in0=zz[:],
                            in1=C["c_gg4"][:].rearrange(
                                "p (t f) -> p t f", t=4), op=OP.mult)
                        zz = zg
                    if not flags["beg0"]:
                        zb = gwk.tile([P, 4, P], BF16, tag="zb")
                        nc.vector.tensor_tensor(
                            out=zb[:], in0=zz[:],
                            in1=C["c_beg4"][:].rearrange(
                                "p (t f) -> p t f", t=4), op=OP.add)
                        zz = zb
                    h2 = gwk.tile([P, 4, P], BF16, tag="h2")
                    nc.scalar.activation(h2[:], zz[:], AF.Relu)
                    # gate = sigmoid(h2 @ Wg2 + b_g2)
                    pg = pp3.tile([P, 512], F32, tag="h")
                    for t in range(4):
                        pth = pp3t.tile([P, P], BF16, tag="t")
                        nc.tensor.transpose(pth[:], h2[:, t, :],
                                            C["c_ident"][:])
                        h2T = gwk.tile([P, P], BF16, tag="h2T")
                        nc.scalar.activation(h2T[:], pth[:], AF.Identity)
                        nc.tensor.matmul(pg[:, t * P:(t + 1) * P], h2T[:],
                                         C["c_Wg2"][:], start=True, stop=True)
                    gsrc = pg[:]
                    if not flags["bg20"]:
                        gp = gwk.tile([P, 512], F32, tag="gp")
                        nc.vector.tensor_tensor(out=gp[:], in0=pg[:],
                                                in1=C["c_bg24"][:], op=OP.add)
                        gsrc = gp[:]
                    gate = gwk.tile([P, 4, P], BF16, tag="gate")
                    nc.scalar.activation(
                        gate[:], gsrc.rearrange("p (t f) -> p t f", t=4),
                        AF.Sigmoid)
                    p_sb = gwk.tile([P, 4, P], BF16, tag="p_sb")
                    nc.vector.tensor_tensor(out=p_sb[:], in0=v2c_nm[:, ws, :],
                                            in1=gate[:], op=OP.mult)
                    # final LN -> f32 out
                    fbst = gwk.tile([P, 4, 6], F32, tag="fbst")
                    fmv = gwk.tile([P, 4, 2], F32, tag="fmv")
                    for t in range(4):
                        nc.vector.bn_stats(fbst[:, t, :], p_sb[:, t, :])
                        nc.vector.bn_aggr(fmv[:, t, :], fbst[:, t, :])
                    fsd = gwk.tile([P, 4], F32, tag="fsd")
                    nc.scalar.activation(fsd[:], fmv[:, :, 1], AF.Sqrt,
                                         bias=C["c_eps"][:])
                    fiv = gwk.tile([P, 4], F32, tag="fiv")
                    nc.vector.reciprocal(fiv[:], fsd[:])
                    fnm = gwk.tile([P, 4], F32, tag="fnm")
                    nc.vector.scalar_tensor_tensor(
                        out=fnm[:], in0=fmv[:, :, 0], scalar=-1.0, in1=fiv[:],
                        op0=OP.mult, op1=OP.mult)
                    y1 = gwk.tile([P, 4, P], F32, tag="y1")
                    nc.vector.tensor_tensor(
                        out=y1[:], in0=p_sb[:],
                        in1=fiv[:].rearrange("p (t o) -> p t o", o=1)
                            .to_broadcast([P, 4, P]), op=OP.mult)
                    y2 = gwk.tile([P, 4, P], F32, tag="y2")
                    nc.vector.tensor_tensor(
                        out=y2[:], in0=y1[:],
                        in1=fnm[:].rearrange("p (t o) -> p t o", o=1)
                            .to_broadcast([P, 4, P]), op=OP.add)
                    yy = y2
                    if not flags["go1"]:
                        y3 = gwk.tile([P, 4, P], F32, tag="y3")
                        nc.vector.tensor_tensor(
                            out=y3[:], in0=yy[:],
                            in1=C["c_go4"][:].rearrange(
                                "p (t f) -> p t f", t=4), op=OP.mult)
                        yy = y3
                    if not flags["bo0"]:
                        y4 = gwk.tile([P, 4, P], F32, tag="y4")
                        nc.vector.tensor_tensor(
                            out=y4[:], in0=yy[:],
                            in1=C["c_bo4"][:].rearrange(
                                "p (t f) -> p t f", t=4), op=OP.add)
                        yy = y4
                    nc.scalar.dma_start(out=d_out.ap()[sl], in_=yy[:])
                if PH == 2:
                    for sl in range(SH_SLAB):
                        ws = slice(sl * 4, sl * 4 + 4)
                        dbg = gwk.tile([P, 4, P], F32, tag="dbg2")
                        nc.vector.tensor_copy(out=dbg[:], in_=v2c_nm[:, ws, :])
                        nc.scalar.dma_start(out=d_out.ap()[sl], in_=dbg[:])

    nc.compile()
    return nc


# ----------------------------------------------------------------------------
# Entry point
# ----------------------------------------------------------------------------

_CACHE = {}


def _get_kernel(cfg, flags):
    key = (cfg.N, cfg.E, cfg.S_SUB, tuple(sorted(flags.items())))
    if key not in _CACHE:
        _CACHE[key] = build_kernel(cfg, flags)
    return _CACHE[key]


def bench_hw(nc, in_maps, iters=32):
    """Build the sharded PJRT callable once; time repeated executions.

    Output buffers are zero-filled ON DEVICE each iteration (no host
    upload in the timed loop).
    """
    import time
    import jax
    from jax.sharding import Mesh, PartitionSpec, NamedSharding
    from jax.experimental.shard_map import shard_map
    import concourse.mybir as mb
    from concourse import bass2jax as b2j

    b2j.install_neuronx_cc_hook()
    n_cores = len(in_maps)
    partition_name = (nc.partition_id_tensor.name
                      if nc.partition_id_tensor else None)
    in_names, out_names, out_avals, zero_outs = [], [], [], []
    for alloc in nc.m.functions[0].allocations:
        if not isinstance(alloc, mb.MemoryLocationSet):
            continue
        name = alloc.memorylocations[0].name
        if alloc.kind == "ExternalInput":
            if name != partition_name:
                in_names.append(name)
        elif alloc.kind == "ExternalOutput":
            out_names.append(name)
            shape = tuple(alloc.tensor_shape)
            dtype = mb.dt.np(alloc.dtype)
            out_avals.append(jax.core.ShapedArray(shape, dtype))
            zero_outs.append(np.zeros(shape, dtype))
    n_params = len(in_names)
    n_outs = len(out_avals)
    in_names.extend(out_names)
    if partition_name is not None:
        in_names.append(partition_name)
    donate = tuple(range(n_params, n_params + n_outs))

    def _body(*args):
        operands = list(args)
        if partition_name is not None:
            operands.append(b2j.partition_id_tensor())
        outs = b2j._bass_exec_p.bind(
            *operands,
            out_avals=tuple(out_avals), in_names=tuple(in_names),
            out_names=tuple(out_names), lowering_input_output_aliases=(),
            sim_require_finite=True, sim_require_nnan=True, nc=nc)
        return tuple(outs)

    devices = jax.devices()[:n_cores]
    mesh = Mesh(np.asarray(devices), ("core",))
    sharded = jax.jit(
        shard_map(_body, mesh=mesh,
                  in_specs=(PartitionSpec("core"),) * (n_params + n_outs),
                  out_specs=(PartitionSpec("core"),) * n_outs,
                  check_rep=False),
        donate_argnums=donate, keep_unused=True)

    concat_in = [
        np.concatenate([np.asarray(in_maps[c][in_names[i]])
                        for c in range(n_cores)], axis=0)
        for i in range(n_params)]
    in_shardings = [NamedSharding(mesh, PartitionSpec("core"))] * n_params
    in_bufs = [jax.device_put(a, s) for a, s in zip(concat_in, in_shardings)]

    import jax.numpy as jnp
    zero_sharding = tuple(
        NamedSharding(mesh, PartitionSpec("core")) for _ in range(n_outs))
    zeros_jit = jax.jit(
        lambda: tuple(
            jnp.zeros((n_cores * z.shape[0], *z.shape[1:]), z.dtype)
            for z in zero_outs),
        out_shardings=zero_sharding)

    def fresh_zeros():
        return list(zeros_jit())

    out_arrs = sharded(*in_bufs, *fresh_zeros())
    jax.block_until_ready(out_arrs)
    results = [
        {name: np.asarray(out_arrs[i]).reshape(n_cores, *out_avals[i].shape)[c]
         for i, name in enumerate(out_names)}
        for c in range(n_cores)]

    t0 = time.perf_counter()
    outs = []
    for _ in range(iters):
        outs.append(sharded(*in_bufs, *fresh_zeros()))
    jax.block_until_ready(outs)
    dt = (time.perf_counter() - t0) / iters
    return results, dt * 1e9


def kernel(**inputs):
    global LAST_EXEC_NS
    N, E = 65536, 262144
    cfg = Cfg(N, E)
    cfg, in_maps, p2o, flags = host_prep(cfg, inputs)
    nc = _get_kernel(cfg, flags)
    if bool(int(os.environ.get("GNN_BENCH", "1"))):
        results, ns = bench_hw(nc, in_maps,
                               iters=int(os.environ.get("GNN_ITERS", "32")))
        LAST_EXEC_NS = ns
    else:
        res = run_bass_kernel_spmd(nc, in_maps, core_ids=list(range(NCORES)))
        results = res.results
        LAST_EXEC_NS = res.exec_time_ns
    NSHARD = cfg.NSHARD
    y_perm = np.concatenate(
        [results[k]["y"].reshape(NSHARD // 512, P, 4, P)
         .transpose(0, 2, 1, 3).reshape(NSHARD, P)
         for k in range(NCORES)], axis=0)
    y = np.empty_like(y_perm)
    y[p2o] = y_perm
    return y.astype(np.float32)


LAST_EXEC_NS = None


# revision 7
# speedup vs baseline: 57.3457x; 1.1847x over previous
"""Trainium2 Bass kernel for the ExplicitV2C GNN layer (GATv2 message passing).

Strategy (8-core SPMD, no collectives):
  * Host: permute nodes into 512 degree-balanced bins of 128 nodes; group
    edges by destination bin; pad each bin to S subtiles of 128 edges.
    Each core owns 64 bins (8192 dst nodes) and all edges targeting them.
  * Device per core:
      Phase 1 (replicated): LLR fusion (Linear+LN+ReLU+mask) over ALL nodes;
        writes the full bf16 x_w table to core-local DRAM (gather source).
      Phase 2 (edges, sharded): batched indirect-DMA gathers of x_w rows
        (2560 rows per DMA op, including each window's own dst nodes), GATv2
        scores with bf16 matmuls, leaky_relu as 0.2*z + 0.8*relu(z),
        segment softmax + weighted aggregation via one-hot matmuls in PSUM.
      Phase 3 (nodes, sharded): degree gate + final LayerNorm; the degree
        embedding term is fetched with a single dma_gather op.
  * Host: reorder the output shards, undo the node permutation.
"""

import os
import sys

sys.path.insert(0, "/opt/trn_rl_repo")

import numpy as np
import ml_dtypes

import concourse.bass as bass
import concourse.bacc as bacc
import concourse.mybir as mybir
import concourse.tile as tile
from concourse.bass import IndirectOffsetOnAxis
from concourse.bass_utils import run_bass_kernel_spmd

F32 = mybir.dt.float32
BF16 = mybir.dt.bfloat16
I32 = mybir.dt.int32
I16 = mybir.dt.int16
AX = mybir.AxisListType
OP = mybir.AluOpType
AF = mybir.ActivationFunctionType

P = 128
NCORES = 8
LN_EPS = 1e-5
SM_EPS = 1e-16
BF = ml_dtypes.bfloat16


class Cfg:
    def __init__(self, N=65536, E=262144, S_SUB=4):
        self.N, self.E, self.S_SUB = N, E, S_SUB
        self.BINS = N // P                       # node bins total (512)
        self.BPC = self.BINS // NCORES           # windows per core (64)
        self.NSHARD = N // NCORES                # nodes per core (8192)
        self.SLOTS = S_SUB * P                   # edge slots per bin
        self.NSLAB = N // 512                    # phase-1 slabs (128)
        self.CH = 16                             # gather chunks per core
        self.WPC = self.BPC // self.CH           # windows per chunk (4)
        self.CPW = S_SUB + 1                     # gather cols per window


# ----------------------------------------------------------------------------
# Host-side preprocessing
# ----------------------------------------------------------------------------

def _balance_bins(deg_in, N, BINS, target):
    """LPT assignment: nodes by in-degree descending onto the lightest bin
    that still has free slots; every bin gets exactly P nodes."""
    import heapq
    order = np.argsort(-deg_in, kind="stable")
    bin_of = np.empty(N, np.int64)
    slot_of = np.empty(N, np.int64)
    heap = [(0, 0, b) for b in range(BINS)]
    heapq.heapify(heap)
    for n in order:
        while True:
            load, cnt, b = heapq.heappop(heap)
            if cnt < P:
                break
        bin_of[n] = b
        slot_of[n] = cnt
        heapq.heappush(heap, (load + int(deg_in[n]), cnt + 1, b))
    loads = np.bincount(bin_of, weights=deg_in, minlength=BINS).astype(np.int64)
    return bin_of, slot_of, loads


def host_prep(cfg, inputs):
    N, E = cfg.N, cfg.E
    BINS, BPC, NSHARD = cfg.BINS, cfg.BPC, cfg.NSHARD

    x = np.asarray(inputs["x"], np.float32)
    ei = np.asarray(inputs["edge_index"])
    src_o = ei[0].astype(np.int64)
    dst_o = ei[1].astype(np.int64)
    ea = np.asarray(inputs["edge_attr"], np.float32)
    ndeg = np.asarray(inputs["node_degrees"]).astype(np.int64)
    llr = np.asarray(inputs["llr_features"], np.float32).reshape(N)
    vmask = np.asarray(inputs["var_node_mask"]).astype(np.float32).reshape(N)

    deg_in = np.bincount(dst_o, minlength=N).astype(np.int64)
    target = -(-E // BINS)
    bin_of, slot_of, loads = _balance_bins(deg_in, N, BINS, target)
    max_load = int(loads.max())
    S = max(1, -(-max_load // P))
    cfg = Cfg(N, E, S)
    SLOTS = cfg.SLOTS
    CH, WPC, CPW = cfg.CH, cfg.WPC, cfg.CPW

    # permuted node id: node o sits at (bin, slot)
    o2p = bin_of * P + slot_of
    p2o = np.argsort(o2p)          # p2o[pid] = original id

    # x_w DRAM table row of permuted node n: n = slab*512 + t*128 + p is
    # stored at row slab*512 + p*4 + t (matches contiguous slab stores)
    n_ids = np.arange(N)
    n_slab = n_ids // 512
    n_t = (n_ids % 512) // P
    n_p = n_ids % P
    row_of_node = n_slab * 512 + n_p * 4 + n_t

    # --- edge arrays grouped by destination bin ---------------------------
    src_p = o2p[src_o]
    dst_pid = o2p[dst_o]
    ebin = dst_pid >> 7
    eslot = dst_pid & 127

    eorder = np.argsort(ebin, kind="stable")
    ebin_s = ebin[eorder]
    starts = np.zeros(BINS + 1, np.int64)
    np.cumsum(np.bincount(ebin_s, minlength=BINS), out=starts[1:])
    rank = np.arange(E) - starts[ebin_s]
    q = ebin_s * SLOTS + rank                 # position in padded layout

    esrc = np.zeros(BINS * SLOTS, np.int64)   # permuted src node id
    eslot_f = np.full(BINS * SLOTS, float(P), np.float32)   # pad slot = P
    eattr = np.zeros((BINS * SLOTS, 8), np.float32)
    esrc[q] = src_p[eorder]
    eslot_f[q] = eslot[eorder].astype(np.float32)
    eattr[q] = ea[eorder]

    # gather row index per edge slot (into the shuffled x_w table layout).
    # Pad slots point at the last table row (positive int16 after re-basing),
    # and each bin's slots are stably partitioned so that positive-row slots
    # come last: the transpose-gather drops trailing NEGATIVE indices, so the
    # final index of every per-window gather op must be non-negative.
    egrow_f = np.full(BINS * SLOTS, N - 1, np.int64)
    filled = np.zeros(BINS * SLOTS, bool)
    filled[q] = True
    egrow_f[q] = row_of_node[src_p[eorder]]
    eg2 = egrow_f.reshape(BINS, SLOTS)
    es2 = eslot_f.reshape(BINS, SLOTS)
    ea2 = eattr.reshape(BINS, SLOTS, 8)
    order2 = np.argsort(eg2 >= N // 2, axis=1, kind="stable")
    eg2 = np.take_along_axis(eg2, order2, axis=1)
    es2 = np.take_along_axis(es2, order2, axis=1)
    ea2 = np.take_along_axis(ea2, order2[:, :, None], axis=1)
    eslot_f = es2.reshape(-1)
    eattr = ea2.reshape(-1, 8)
    egrow = eg2.reshape(BINS, S, P)                   # [win_glob, j, p]

    # per-core transpose-gather indices: int16 = table_row - N/2 (sign trick
    # extends the addressable range to 65536 rows).  Position i = col*128 + e;
    # the CPW cols of window w are [own nodes, edge subtile 0..S-1].
    NPC = WPC * CPW * P                           # idx positions per chunk
    idx_g = np.zeros((NCORES, CH, P, NPC // 16), np.int16)
    half = N // 2
    for c in range(NCORES):
        for ch in range(CH):
            unw = np.zeros(NPC, np.int64)
            for wdx in range(WPC):
                wg = c * BPC + ch * WPC + wdx     # global bin
                base = wdx * CPW * P
                own_nodes = wg * P + np.arange(P) # permuted ids of own bin
                unw[base:base + P] = row_of_node[own_nodes]
                for j in range(S):
                    unw[base + (1 + j) * P:base + (2 + j) * P] = egrow[wg, j]
            w16 = (unw - half).astype(np.int16).reshape(NPC // 16, 16).T
            idx_g[c, ch] = np.tile(w16, (8, 1))

    eslot_r = eslot_f.reshape(NCORES, BPC * S, P)
    dst_col = eslot_r.transpose(0, 2, 1).copy()               # [c, p, col]
    eattr_r = eattr.reshape(NCORES, BPC * S, P, 8)
    ea_t = eattr_r.transpose(0, 3, 1, 2).reshape(
        NCORES, 8, BPC * S * P).astype(BF)                    # [c, 8, col*p]

    # --- node arrays (full, replicated) -----------------------------------
    xp = x[p2o]                                              # [N, HID]
    x_t_full = np.ascontiguousarray(xp.T.astype(BF))         # [128, N]
    # interleaved rows: [slab, p, t, f], node n = slab*512 + t*128 + p
    xr4 = np.ascontiguousarray(
        xp.reshape(cfg.NSLAB, 4, P, P).transpose(0, 2, 1, 3).astype(BF))
    # llr per node: [p, slab, t]
    llr4 = np.ascontiguousarray(
        llr[p2o].reshape(cfg.NSLAB, 4, P).transpose(2, 0, 1).astype(BF))
    # mask: [p, slab, t]
    m_all = np.ascontiguousarray(
        vmask[p2o].reshape(cfg.NSLAB, 4, P).transpose(2, 0, 1).astype(BF))

    # degree gather indices (int16), wrap order, replicated to 128 parts
    degc = np.clip(ndeg, 0, 99)[p2o].reshape(NCORES, NSHARD).astype(np.int16)
    deg_wrap = degc.reshape(NCORES, NSHARD // 16, 16).transpose(0, 2, 1)
    deg_rep = np.ascontiguousarray(np.tile(deg_wrap, (1, 8, 1)))  # [c,128,S]

    # --- weights -----------------------------------------------------------
    w = {k: np.asarray(v, np.float32) for k, v in inputs.items()
         if k not in ("x", "edge_index", "edge_attr", "node_degrees",
                      "llr_features", "var_node_mask")}
    att = w["att"]                                           # [4,128]

    def bcast_row(v, reps):                                  # [P, reps*128]
        return np.ascontiguousarray(
            np.broadcast_to(np.tile(v, reps)[None, :], (P, reps * P)))

    flags = {
        "gf1": bool(np.allclose(w["g_f"], 1.0)),
        "bef0": bool(np.allclose(w["be_f"], 0.0)),
        "gg1": bool(np.allclose(w["g_g"], 1.0)),
        "beg0": bool(np.allclose(w["be_g"], 0.0)),
        "bg20": bool(np.allclose(w["b_g2"], 0.0)),
        "go1": bool(np.allclose(w["g_o"], 1.0)),
        "bo0": bool(np.allclose(w["b_o"], 0.0)),
    }

    consts = {
        "c_Wfx": w["W_f"][:P].astype(BF),                    # [128,128]
        "c_wfl4": np.ascontiguousarray(np.broadcast_to(
            np.tile(w["W_f"][P], 4)[None, :], (P, 512)).astype(BF)),
        "c_bfc": np.ascontiguousarray(
            w["b_f"].reshape(P, 1).astype(np.float32)),
        "c_eps": np.full((P, 1), LN_EPS, np.float32),
        "c_Wl": w["W_l"].astype(BF),                         # [128,512]
        "c_Wr": w["W_r"].astype(BF),
        "c_We": w["W_e"].astype(BF),                         # [8,512]
        "c_attb": np.ascontiguousarray(
            np.broadcast_to(att.reshape(1, 512), (P, 512)).astype(BF)),
        "c_iota": np.ascontiguousarray(
            np.broadcast_to(np.arange(P, dtype=np.float32)[None, :], (P, P))),
        "c_ident": np.eye(P, dtype=BF),
        "c_Wg1x": w["W_g1"][:P].astype(BF),                  # [128,128]
        "c_Wg2": w["W_g2"].astype(BF),
        "c_Td": (w["deg_emb"] @ w["W_g1"][P:P + 16]
                 + w["b_g1"][None, :]).astype(np.float32),   # [100,128]
    }
    if not flags["gf1"]:
        consts["c_gf4"] = bcast_row(w["g_f"], 4).astype(BF)
    if not flags["bef0"]:
        consts["c_bef4"] = bcast_row(w["be_f"], 4).astype(BF)
    if not flags["gg1"]:
        consts["c_gg4"] = bcast_row(w["g_g"], 4).astype(BF)
    if not flags["beg0"]:
        consts["c_beg4"] = bcast_row(w["be_g"], 4).astype(BF)
    if not flags["bg20"]:
        consts["c_bg24"] = bcast_row(w["b_g2"], 4).astype(np.float32)
    if not flags["go1"]:
        consts["c_go4"] = bcast_row(w["g_o"], 4).astype(np.float32)
    if not flags["bo0"]:
        consts["c_bo4"] = bcast_row(w["b_o"], 4).astype(np.float32)
    consts = {k: np.ascontiguousarray(v) for k, v in consts.items()}

    in_maps = []
    for k in range(NCORES):
        m = {
            "x_t": x_t_full, "xr4": xr4, "llr4": llr4, "m_all": m_all,
            "idx_g": np.ascontiguousarray(idx_g[k]),
            "dst_col": np.ascontiguousarray(dst_col[k]),
            "ea_t": np.ascontiguousarray(ea_t[k]),
            "deg_w": deg_rep[k],
        }
        m.update(consts)
        in_maps.append(m)
    return cfg, in_maps, p2o, flags


# ----------------------------------------------------------------------------
# Device kernel
# ----------------------------------------------------------------------------

def build_kernel(cfg, flags):
    PH = int(os.environ.get("GNN_PH", "3"))
    N, S = cfg.N, cfg.S_SUB
    BPC, NSHARD = cfg.BPC, cfg.NSHARD
    NSLAB, CH, WPC, CPW = cfg.NSLAB, cfg.CH, cfg.WPC, cfg.CPW
    SH_SLAB = NSHARD // 512                      # output slabs per core (16)
    COLS = BPC * S                               # edge subtile columns (256)

    nc = bacc.Bacc("TRN2", target_bir_lowering=False, debug=False,
                   num_devices=NCORES)

    # ---- I/O ----
    d_xt = nc.dram_tensor("x_t", [P, N], BF16, kind="ExternalInput")
    d_xr4 = nc.dram_tensor("xr4", [NSLAB, P, 4, P], BF16, kind="ExternalInput")
    d_llr = nc.dram_tensor("llr4", [P, NSLAB, 4], BF16, kind="ExternalInput")
    d_m = nc.dram_tensor("m_all", [P, NSLAB, 4], BF16, kind="ExternalInput")
    d_idx = nc.dram_tensor("idx_g", [CH, P, WPC * CPW * 8], I16,
                           kind="ExternalInput")
    d_dst = nc.dram_tensor("dst_col", [P, COLS], F32, kind="ExternalInput")
    d_eat = nc.dram_tensor("ea_t", [8, COLS * P], BF16, kind="ExternalInput")
    d_deg = nc.dram_tensor("deg_w", [P, NSHARD // 16], I16,
                           kind="ExternalInput")
    d_out = nc.dram_tensor("y", [SH_SLAB, P, 4, P], F32, kind="ExternalOutput")

    cshape = {
        "c_Wfx": ([P, P], BF16), "c_wfl4": ([P, 512], BF16),
        "c_bfc": ([P, 1], F32), "c_eps": ([P, 1], F32),
        "c_Wl": ([P, 512], BF16), "c_Wr": ([P, 512], BF16),
        "c_We": ([8, 512], BF16), "c_attb": ([P, 512], BF16),
        "c_iota": ([P, P], F32), "c_ident": ([P, P], BF16),
        "c_Wg1x": ([P, P], BF16), "c_Wg2": ([P, P], BF16),
        "c_Td": ([100, P], F32),
    }
    for nm, fl, dt in (("c_gf4", "gf1", BF16), ("c_bef4", "bef0", BF16),
                       ("c_gg4", "gg1", BF16), ("c_beg4", "beg0", BF16),
                       ("c_bg24", "bg20", F32), ("c_go4", "go1", F32),
                       ("c_bo4", "bo0", F32)):
        if not flags[fl]:
            cshape[nm] = ([P, 512], dt)
    d_c = {k: nc.dram_tensor(k, sh, dt, kind="ExternalInput")
           for k, (sh, dt) in cshape.items()}

    # gather table with a shadow copy of the first half appended: the
    # transpose-gather reads rows [0,N) via int16 idx relative to base N/2,
    # but its declared AP is rows [N/2, 2N) -- the shadow writes make every
    # phase-1 store overlap that range so the dep tracker orders them.
    d_xw = nc.dram_tensor("xw_tab", [2 * N, P], BF16)
    d_td = nc.dram_tensor("td_tab", [100, P], F32)

    with tile.TileContext(nc) as tc:
        with (
            tc.tile_pool(name="const", bufs=1) as cpool,
            tc.tile_pool(name="resid", bufs=1) as rpool,
        ):
            C = {}
            for k, (sh, dt) in cshape.items():
                C[k] = cpool.tile(sh, dt, tag=k, name=f"const_{k}")
                nc.sync.dma_start(out=C[k][:], in_=d_c[k].ap())
            nc.sync.dma_start(out=d_td.ap(), in_=C["c_Td"][:])

            llr_s = rpool.tile([P, NSLAB, 4], BF16, tag="llr")
            nc.sync.dma_start(out=llr_s[:], in_=d_llr.ap())
            m_s = rpool.tile([P, NSLAB, 4], BF16, tag="m")
            nc.sync.dma_start(out=m_s[:], in_=d_m.ap())
            dst_t = rpool.tile([P, COLS], F32, tag="dst")
            nc.sync.dma_start(out=dst_t[:], in_=d_dst.ap())
            eat_t = rpool.tile([8, COLS * P], BF16, tag="eat")
            nc.sync.dma_start(out=eat_t[:], in_=d_eat.ap())
            deg_t = rpool.tile([P, NSHARD // 16], I16, tag="deg")
            nc.sync.dma_start(out=deg_t[:], in_=d_deg.ap())

            v2c_nm = rpool.tile([P, BPC, P], BF16, tag="v2c")  # [p, w, f]
            dterm = rpool.tile([P, BPC, P], F32, tag="dterm")

            # degree-embedding term for all own nodes (1024-idx chunks --
            # a single instruction's descriptors must fit the SWDGE ring)
            for g in range(NSHARD // 1024):
                nc.gpsimd.dma_gather(
                    out_ap=dterm[:, g * 8:(g + 1) * 8, :], in_ap=d_td.ap(),
                    idxs_ap=deg_t[:, g * 64:(g + 1) * 64],
                    num_idxs=1024, num_idxs_reg=1024, elem_size=P,
                    transpose=False)

            # ================= Phase 1: LLR fusion (replicated) =============
            with (
                tc.tile_pool(name="p1mm", bufs=2, space="PSUM") as pp1,
                tc.tile_pool(name="p1tr", bufs=2, space="PSUM") as pp1t,
                tc.tile_pool(name="p1in", bufs=3) as sb1i,
                tc.tile_pool(name="p1wk", bufs=2) as sb1,
            ):
                for s in range(NSLAB):
                    ns = slice(s * 512, (s + 1) * 512)
                    xt_sl = sb1i.tile([P, 512], BF16, tag="xt")
                    nc.sync.dma_start(out=xt_sl[:], in_=d_xt.ap()[:, ns])
                    py = pp1.tile([P, 512], F32, tag="y")
                    nc.tensor.matmul(py[:], C["c_Wfx"][:], xt_sl[:],
                                     start=True, stop=True)
                    ytT = sb1.tile([P, 512], BF16, tag="ytT")
                    nc.scalar.activation(ytT[:], py[:], AF.Identity,
                                         bias=C["c_bfc"][:])
                    ptq = pp1t.tile([P, 512], BF16, tag="tr")
                    for t in range(4):
                        qs = slice(t * P, (t + 1) * P)
                        nc.tensor.transpose(ptq[:, qs], ytT[:, qs],
                                            C["c_ident"][:])
                    wl4 = sb1.tile([P, 4, P], BF16, tag="wl4")
                    nc.vector.tensor_tensor(
                        out=wl4[:],
                        in0=C["c_wfl4"][:].rearrange("p (t f) -> p t f", t=4),
                        in1=llr_s[:, s, :].rearrange("p (t o) -> p t o", o=1)
                            .to_broadcast([P, 4, P]), op=OP.mult)
                    yr = sb1.tile([P, 4, P], BF16, tag="yr")
                    nc.vector.tensor_tensor(
                        out=yr[:], in0=ptq[:].rearrange("p (t f) -> p t f", t=4),
                        in1=wl4[:], op=OP.add)
                    bst = sb1.tile([P, 4, 6], F32, tag="bst")
                    mv = sb1.tile([P, 4, 2], F32, tag="mv")
                    for t in range(4):
                        nc.vector.bn_stats(bst[:, t, :], yr[:, t, :])
                        nc.vector.bn_aggr(mv[:, t, :], bst[:, t, :])
                    sd4 = sb1.tile([P, 4], F32, tag="sd4")
                    nc.scalar.activation(sd4[:], mv[:, :, 1], AF.Sqrt,
                                         bias=C["c_eps"][:])
                    iv4 = sb1.tile([P, 4], F32, tag="iv4")
                    nc.vector.reciprocal(iv4[:], sd4[:])
                    nm4 = sb1.tile([P, 4], F32, tag="nm4")
                    nc.vector.scalar_tensor_tensor(
                        out=nm4[:], in0=mv[:, :, 0], scalar=-1.0, in1=iv4[:],
                        op0=OP.mult, op1=OP.mult)
                    t1 = sb1.tile([P, 4, P], BF16, tag="t1")
                    nc.vector.tensor_tensor(
                        out=t1[:], in0=yr[:],
                        in1=iv4[:].rearrange("p (t o) -> p t o", o=1)
                            .to_broadcast([P, 4, P]), op=OP.mult)
                    t2 = sb1.tile([P, 4, P], BF16, tag="t2")
                    nc.vector.tensor_tensor(
                        out=t2[:], in0=t1[:],
                        in1=nm4[:].rearrange("p (t o) -> p t o", o=1)
                            .to_broadcast([P, 4, P]), op=OP.add)
                    zz = t2
                    if not flags["gf1"]:
                        zg = sb1.tile([P, 4, P], BF16, tag="zg")
                        nc.vector.tensor_tensor(
                            out=zg[:], in0=zz[:],
                            in1=C["c_gf4"][:].rearrange(
                                "p (t f) -> p t f", t=4), op=OP.mult)
                        zz = zg
                    if not flags["bef0"]:
                        zb = sb1.tile([P, 4, P], BF16, tag="zb")
                        nc.vector.tensor_tensor(
                            out=zb[:], in0=zz[:],
                            in1=C["c_bef4"][:].rearrange(
                                "p (t f) -> p t f", t=4), op=OP.add)
                        zz = zb
                    fu = sb1.tile([P, 4, P], BF16, tag="fu")
                    nc.scalar.activation(fu[:], zz[:], AF.Relu)
                    xr_sl = sb1i.tile([P, 4, P], BF16, tag="xr")
                    nc.scalar.dma_start(out=xr_sl[:], in_=d_xr4.ap()[s])
                    d1 = sb1.tile([P, 4, P], BF16, tag="d1")
                    nc.vector.tensor_tensor(out=d1[:], in0=fu[:], in1=xr_sl[:],
                                            op=OP.subtract)
                    dm = sb1.tile([P, 4, P], BF16, tag="dm")
                    nc.vector.tensor_tensor(
                        out=dm[:], in0=d1[:],
                        in1=m_s[:, s, :].rearrange("p (t o) -> p t o", o=1)
                            .to_broadcast([P, 4, P]), op=OP.mult)
                    xw_sl = sb1.tile([P, 4, P], BF16, tag="xw")
                    nc.vector.tensor_tensor(out=xw_sl[:], in0=dm[:],
                                            in1=xr_sl[:], op=OP.add)
                    nc.sync.dma_start(
                        out=d_xw.ap()[ns, :].rearrange(
                            "(pp t) f -> pp t f", t=4),
                        in_=xw_sl[:])
                    if s < NSLAB // 2:
                        sh = slice(N + s * 512, N + (s + 1) * 512)
                        nc.scalar.dma_start(
                            out=d_xw.ap()[sh, :].rearrange(
                                "(pp t) f -> pp t f", t=4),
                            in_=xw_sl[:])
                    if PH == 1 and s < SH_SLAB:
                        dbg = sb1.tile([P, 4, P], F32, tag="dbg")
                        nc.vector.tensor_copy(out=dbg[:], in_=xw_sl[:])
                        nc.scalar.dma_start(out=d_out.ap()[s], in_=dbg[:])

            # ================= Phase 2: edges ===============================
            with (
                tc.tile_pool(name="pz", bufs=2, space="PSUM") as ppz,
                tc.tile_pool(name="pxl", bufs=2, space="PSUM") as ppxl,
                tc.tile_pool(name="po4", bufs=1, space="PSUM") as ppo4,
                tc.tile_pool(name="psm", bufs=1, space="PSUM") as ppsm,
                tc.tile_pool(name="ptr", bufs=2, space="PSUM") as pptr,
                tc.tile_pool(name="e_in", bufs=2) as ein,
                tc.tile_pool(name="e_wk", bufs=3) as ewk,
                tc.tile_pool(name="e_w2", bufs=2) as ewk2,
            ):
                NPC = WPC * CPW * P
                NPW = CPW * P                      # idx positions per window
                for ch in range(CH if PH >= 2 else 0):
                    idx_t = ein.tile([P, NPC // 16], I16, tag="idx")
                    nc.sync.dma_start(out=idx_t[:], in_=d_idx.ap()[ch])
                    # feature-major gather: column i holds x_w of idx i
                    xg_t = ein.tile([P, 1, NPC], BF16, tag="xg")
                    for wdx in range(WPC):
                        nc.gpsimd.dma_gather(
                            out_ap=xg_t[:, :, wdx * NPW:(wdx + 1) * NPW],
                            in_ap=d_xw.ap()[N // 2:2 * N, :],
                            idxs_ap=idx_t[:, wdx * (NPW // 16):
                                          (wdx + 1) * (NPW // 16)],
                            num_idxs=NPW, num_idxs_reg=NPW,
                            elem_size=P, transpose=True)

                    for wdx in range(WPC):
                        win = ch * WPC + wdx
                        base = wdx * CPW * P
                        # xr tile for this window's own (dst) nodes
                        xwT = xg_t[:, 0, base:base + P]
                        pxr = ppz.tile([P, 512], F32, tag="z")
                        nc.tensor.matmul(pxr[:], xwT, C["c_Wr"][:],
                                         start=True, stop=True)
                        xr_sb = ewk2.tile([P, 512], BF16, tag="xr")
                        nc.vector.tensor_copy(out=xr_sb[:], in_=pxr[:])

                        pden = ppsm.tile([P, 4], F32, tag="sm")
                        po4 = ppo4.tile([P, 512], F32, tag="o4")

                        for j in range(S):
                            st = win * S + j
                            # one-hot S [e, d] and its transpose
                            S_sb = ewk.tile([P, P], BF16, tag="S")
                            nc.vector.tensor_tensor(
                                out=S_sb[:],
                                in0=dst_t[:, st:st + 1].to_broadcast([P, P]),
                                in1=C["c_iota"][:], op=OP.is_equal)
                            pts = pptr.tile([P, P], BF16, tag="tr")
                            nc.tensor.transpose(pts[:], S_sb[:],
                                                C["c_ident"][:])
                            st_sb = ewk.tile([P, P], BF16, tag="st")
                            nc.scalar.activation(st_sb[:], pts[:], AF.Identity)
                            # gathered x_w[src] columns (feature-major)
                            xgT = xg_t[:, 0, base + (1 + j) * P:
                                       base + (2 + j) * P]

                            ea_sl = eat_t[:, st * P:(st + 1) * P]
                            pz = ppz.tile([P, 512], F32, tag="z")
                            pxl = ppxl.tile([P, 512], F32, tag="xl")
                            nc.tensor.matmul(pz[:], xgT, C["c_Wl"][:],
                                             start=True, stop=False)
                            nc.tensor.matmul(pxl[:], xgT, C["c_Wl"][:],
                                             start=True, stop=True)
                            nc.tensor.matmul(pz[:], st_sb[:], xr_sb[:],
                                             start=False, stop=False)
                            nc.tensor.matmul(pz[:], ea_sl, C["c_We"][:],
                                             start=False, stop=True)

                            # leaky = 0.2*z + 0.8*relu(z)
                            r_sb = ewk.tile([P, 512], BF16, tag="r")
                            nc.scalar.activation(r_sb[:], pz[:], AF.Relu,
                                                 scale=0.8)
                            lk = ewk.tile([P, 512], BF16, tag="lk")
                            nc.vector.scalar_tensor_tensor(
                                out=lk[:], in0=pz[:], scalar=0.2, in1=r_sb[:],
                                op0=OP.mult, op1=OP.add)
                            # alpha[e,h] = sum_c lk*att
                            zat = ewk.tile([P, 512], BF16, tag="zat")
                            nc.vector.tensor_tensor(out=zat[:], in0=lk[:],
                                                    in1=C["c_attb"][:],
                                                    op=OP.mult)
                            alpha = ewk.tile([P, 4], F32, tag="alpha")
                            nc.vector.reduce_sum(
                                out=alpha[:],
                                in_=zat[:].rearrange("p (h c) -> p h c", h=4),
                                axis=AX.X)
                            au = ewk.tile([P, 4], BF16, tag="au")
                            nc.scalar.activation(au[:], alpha[:], AF.Exp)
                            nc.tensor.matmul(pden[:], S_sb[:], au[:],
                                             start=(j == 0), stop=(j == S - 1))
                            # xl scaled by per-edge attention (per head)
                            xla = ewk.tile([P, 4, P], BF16, tag="xla")
                            nc.vector.tensor_tensor(
                                out=xla[:],
                                in0=pxl[:].rearrange("p (h f) -> p h f", h=4),
                                in1=au[:].rearrange("p (h o) -> p h o", o=1)
                                    .to_broadcast([P, 4, P]), op=OP.mult)
                            nc.tensor.matmul(
                                po4[:], S_sb[:],
                                xla[:].rearrange("p h f -> p (h f)"),
                                start=(j == 0), stop=(j == S - 1))
                        # normalize + head mean -> v2c (node-major)
                        dv = ewk.tile([P, 4], F32, tag="dv")
                        nc.vector.tensor_scalar(out=dv[:], in0=pden[:],
                                                scalar1=SM_EPS, scalar2=None,
                                                op0=OP.add)
                        iv = ewk.tile([P, 4], F32, tag="iv")
                        nc.vector.reciprocal(iv[:], dv[:])
                        nc.vector.tensor_scalar(out=iv[:], in0=iv[:],
                                                scalar1=0.25, scalar2=None,
                                                op0=OP.mult)
                        vsl = v2c_nm[:, win, :]
                        nc.vector.tensor_scalar(
                            out=vsl, in0=po4[:, 0:P], scalar1=iv[:, 0:1],
                            scalar2=None, op0=OP.mult)
                        for h in range(1, 4):
                            hs = slice(h * P, (h + 1) * P)
                            nc.vector.scalar_tensor_tensor(
                                out=vsl, in0=po4[:, hs], scalar=iv[:, h:h + 1],
                                in1=vsl, op0=OP.mult, op1=OP.add)

            # ================= Phase 3: degree gate + final LN ==============
            with (
                tc.tile_pool(name="p3a", bufs=2, space="PSUM") as pp3,
                tc.tile_pool(name="p3t", bufs=2, space="PSUM") as pp3t,
                tc.tile_pool(name="g_wk", bufs=2) as gwk,
            ):
                for sl in range(SH_SLAB if PH >= 3 else 0):
                    ws = slice(sl * 4, sl * 4 + 4)
                    # h = v2c @ Wg1x + dterm (deg-emb term incl. b_g1)
                    ph = pp3.tile([P, 512], F32, tag="h")
                    for t in range(4):
                        win = sl * 4 + t
                        ptv = pp3t.tile([P, P], BF16, tag="t")
                        nc.tensor.transpose(ptv[:], v2c_nm[:, win, :],
                                            C["c_ident"][:])
                        v2cT = gwk.tile([P, P], BF16, tag="v2cT")
                        nc.scalar.activation(v2cT[:], ptv[:], AF.Identity)
                        nc.tensor.matmul(ph[:, t * P:(t + 1) * P], v2cT[:],
                                         C["c_Wg1x"][:], start=True, stop=True)
                    h_sb = gwk.tile([P, 4, P], BF16, tag="h_sb")
                    nc.vector.tensor_tensor(
                        out=h_sb[:],
                        in0=ph[:].rearrange("p (t f) -> p t f", t=4),
                        in1=dterm[:, ws, :], op=OP.add)
                    bst = gwk.tile([P, 4, 6], F32, tag="bst")
                    mv = gwk.tile([P, 4, 2], F32, tag="mv")
                    for t in range(4):
                        nc.vector.bn_stats(bst[:, t, :], h_sb[:, t, :])
                        nc.vector.bn_aggr(mv[:, t, :], bst[:, t, :])
                    sd4 = gwk.tile([P, 4], F32, tag="sd4")
                    nc.scalar.activation(sd4[:], mv[:, :, 1], AF.Sqrt,
                                         bias=C["c_eps"][:])
                    iv4 = gwk.tile([P, 4], F32, tag="iv4")
                    nc.vector.reciprocal(iv4[:], sd4[:])
                    nm4 = gwk.tile([P, 4], F32, tag="nm4")
                    nc.vector.scalar_tensor_tensor(
                        out=nm4[:], in0=mv[:, :, 0], scalar=-1.0, in1=iv4[:],
                        op0=OP.mult, op1=OP.mult)
                    t1 = gwk.tile([P, 4, P], BF16, tag="t1")
                    nc.vector.tensor_tensor(
                        out=t1[:], in0=h_sb[:],
                        in1=iv4[:].rearrange("p (t o) -> p t o", o=1)
                            .to_broadcast([P, 4, P]), op=OP.mult)
                    t2 = gwk.tile([P, 4, P], BF16, tag="t2")
                    nc.vector.tensor_tensor(
                        out=t2[:], in0=t1[:],
                        in1=nm4[:].rearrange("p (t o) -> p t o", o=1)
                            .to_broadcast([P, 4, P]), op=OP.add)
                    zz = t2
                    if not flags["gg1"]:
                        zg = gwk.tile([P, 4, P], BF16, tag="zg")
                        nc.vector.tensor_tensor(
                            out=zg[:], in0=zz[:],
                            in1=C["c_gg4"][:].rearrange(
                                "p (t f) -> p t f", t=4), op=OP.mult)
                        zz = zg
                    if not flags["beg0"]:
                        zb = gwk.tile([P, 4, P], BF16, tag="zb")
                        nc.vector.tensor_tensor(
                            out=zb[:], in0=zz[:],
                            in1=C["c_beg4"][:].rearrange(
                                "p (t f) -> p t f", t=4), op=OP.add)
                        zz = zb
                    h2 = gwk.tile([P, 4, P], BF16, tag="h2")
                    nc.scalar.activation(h2[:], zz[:], AF.Relu)
                    # gate = sigmoid(h2 @ Wg2 + b_g2)
                    pg = pp3.tile([P, 512], F32, tag="h")
                    for t in range(4):
                        pth = pp3t.tile([P, P], BF16, tag="t")
                        nc.tensor.transpose(pth[:], h2[:, t, :],
                                            C["c_ident"][:])
                        h2T = gwk.tile([P, P], BF16, tag="h2T")
                        nc.scalar.activation(h2T[:], pth[:], AF.Identity)
                        nc.tensor.matmul(pg[:, t * P:(t + 1) * P], h2T[:],
                                         C["c_Wg2"][:], start=True, stop=True)
                    gsrc = pg[:]
                    if not flags["bg20"]:
                        gp = gwk.tile([P, 512], F32, tag="gp")
                        nc.vector.tensor_tensor(out=gp[:], in0=pg[:],
                                                in1=C["c_bg24"][:], op=OP.add)
                        gsrc = gp[:]
                    gate = gwk.tile([P, 4, P], BF16, tag="gate")
                    nc.scalar.activation(
                        gate[:], gsrc.rearrange("p (t f) -> p t f", t=4),
                        AF.Sigmoid)
                    p_sb = gwk.tile([P, 4, P], BF16, tag="p_sb")
                    nc.vector.tensor_tensor(out=p_sb[:], in0=v2c_nm[:, ws, :],
                                            in1=gate[:], op=OP.mult)
                    # final LN -> f32 out
                    fbst = gwk.tile([P, 4, 6], F32, tag="fbst")
                    fmv = gwk.tile([P, 4, 2], F32, tag="fmv")
                    for t in range(4):
                        nc.vector.bn_stats(fbst[:, t, :], p_sb[:, t, :])
                        nc.vector.bn_aggr(fmv[:, t, :], fbst[:, t, :])
                    fsd = gwk.tile([P, 4], F32, tag="fsd")
                    nc.scalar.activation(fsd[:], fmv[:, :, 1], AF.Sqrt,
                                         bias=C["c_eps"][:])
                    fiv = gwk.tile([P, 4], F32, tag="fiv")
                    nc.vector.reciprocal(fiv[:], fsd[:])
                    fnm = gwk.tile([P, 4], F32, tag="fnm")
                    nc.vector.scalar_tensor_tensor(
                        out=fnm[:], in0=fmv[:, :, 0], scalar=-1.0, in1=fiv[:],
                        op0=OP.mult, op1=OP.mult)
                    y1 = gwk.tile([P, 4, P], F32, tag="y1")
                    nc.vector.tensor_tensor(
                        out=y1[:], in0=p_sb[:],
                        in1=fiv[:].rearrange("p (t o) -> p t o", o=1)
                            .to_broadcast([P, 4, P]), op=OP.mult)
                    y2 = gwk.tile([P, 4, P], F32, tag="y2")
                    nc.vector.tensor_tensor(
                        out=y2[:], in0=y1[:],
                        in1=fnm[:].rearrange("p (t o) -> p t o", o=1)
                            .to_broadcast([P, 4, P]), op=OP.add)
                    yy = y2
                    if not flags["go1"]:
                        y3 = gwk.tile([P, 4, P], F32, tag="y3")
                        nc.vector.tensor_tensor(
                            out=y3[:], in0=yy[:],
                            in1=C["c_go4"][:].rearrange(
                                "p (t f) -> p t f", t=4), op=OP.mult)
                        yy = y3
                    if not flags["bo0"]:
                        y4 = gwk.tile([P, 4, P], F32, tag="y4")
                        nc.vector.tensor_tensor(
                            out=y4[:], in0=yy[:],
                            in1=C["c_bo4"][:].rearrange(
                                "p (t f) -> p t f", t=4), op=OP.add)
                        yy = y4
                    nc.scalar.dma_start(out=d_out.ap()[sl], in_=yy[:])
                if PH == 2:
                    for sl in range(SH_SLAB):
                        ws = slice(sl * 4, sl * 4 + 4)
                        dbg = gwk.tile([P, 4, P], F32, tag="dbg2")
                        nc.vector.tensor_copy(out=dbg[:], in_=v2c_nm[:, ws, :])
                        nc.scalar.dma_start(out=d_out.ap()[sl], in_=dbg[:])

    nc.compile()
    return nc


# ----------------------------------------------------------------------------
# Entry point
# ----------------------------------------------------------------------------

_CACHE = {}


def _get_kernel(cfg, flags):
    key = (cfg.N, cfg.E, cfg.S_SUB, tuple(sorted(flags.items())))
    if key not in _CACHE:
        _CACHE[key] = build_kernel(cfg, flags)
    return _CACHE[key]


def bench_hw(nc, in_maps, iters=32):
    """Build the sharded PJRT callable once; time repeated executions.

    Output buffers are zero-filled ON DEVICE each iteration (no host
    upload in the timed loop).
    """
    import time
    import jax
    from jax.sharding import Mesh, PartitionSpec, NamedSharding
    from jax.experimental.shard_map import shard_map
    import concourse.mybir as mb
    from concourse import bass2jax as b2j

    b2j.install_neuronx_cc_hook()
    n_cores = len(in_maps)
    partition_name = (nc.partition_id_tensor.name
                      if nc.partition_id_tensor else None)
    in_names, out_names, out_avals, zero_outs = [], [], [], []
    for alloc in nc.m.functions[0].allocations:
        if not isinstance(alloc, mb.MemoryLocationSet):
            continue
        name = alloc.memorylocations[0].name
        if alloc.kind == "ExternalInput":
            if name != partition_name:
                in_names.append(name)
        elif alloc.kind == "ExternalOutput":
            out_names.append(name)
            shape = tuple(alloc.tensor_shape)
            dtype = mb.dt.np(alloc.dtype)
            out_avals.append(jax.core.ShapedArray(shape, dtype))
            zero_outs.append(np.zeros(shape, dtype))
    n_params = len(in_names)
    n_outs = len(out_avals)
    in_names.extend(out_names)
    if partition_name is not None:
        in_names.append(partition_name)
    donate = tuple(range(n_params, n_params + n_outs))

    chain = max(1, int(os.environ.get("GNN_CHAIN", "1")))

    def _body(*args):
        ins = list(args[:n_params])
        outs = list(args[n_params:])
        # chain several executions per dispatch; the (fully overwritten)
        # output operands thread through so XLA cannot CSE the calls
        for _ in range(chain):
            operands = ins + outs
            if partition_name is not None:
                operands.append(b2j.partition_id_tensor())
            outs = list(b2j._bass_exec_p.bind(
                *operands,
                out_avals=tuple(out_avals), in_names=tuple(in_names),
                out_names=tuple(out_names), lowering_input_output_aliases=(),
                sim_require_finite=True, sim_require_nnan=True, nc=nc))
        return tuple(outs)

    devices = jax.devices()[:n_cores]
    mesh = Mesh(np.asarray(devices), ("core",))
    sharded = jax.jit(
        shard_map(_body, mesh=mesh,
                  in_specs=(PartitionSpec("core"),) * (n_params + n_outs),
                  out_specs=(PartitionSpec("core"),) * n_outs,
                  check_rep=False),
        donate_argnums=donate, keep_unused=True)

    concat_in = [
        np.concatenate([np.asarray(in_maps[c][in_names[i]])
                        for c in range(n_cores)], axis=0)
        for i in range(n_params)]
    in_shardings = [NamedSharding(mesh, PartitionSpec("core"))] * n_params
    in_bufs = [jax.device_put(a, s) for a, s in zip(concat_in, in_shardings)]

    import jax.numpy as jnp
    zero_sharding = tuple(
        NamedSharding(mesh, PartitionSpec("core")) for _ in range(n_outs))
    zeros_jit = jax.jit(
        lambda: tuple(
            jnp.zeros((n_cores * z.shape[0], *z.shape[1:]), z.dtype)
            for z in zero_outs),
        out_shardings=zero_sharding)

    def fresh_zeros():
        return list(zeros_jit())

    out_arrs = sharded(*in_bufs, *fresh_zeros())
    jax.block_until_ready(out_arrs)
    results = [
        {name: np.asarray(out_arrs[i]).reshape(n_cores, *out_avals[i].shape)[c]
         for i, name in enumerate(out_names)}
        for c in range(n_cores)]

    t0 = time.perf_counter()
    outs = []
    for _ in range(iters):
        outs.append(sharded(*in_bufs, *fresh_zeros()))
    jax.block_until_ready(outs)
    dt = (time.perf_counter() - t0) / (iters * chain)
    return results, dt * 1e9


def kernel(**inputs):
    global LAST_EXEC_NS
    N, E = 65536, 262144
    cfg = Cfg(N, E)
    cfg, in_maps, p2o, flags = host_prep(cfg, inputs)
    nc = _get_kernel(cfg, flags)
    if bool(int(os.environ.get("GNN_BENCH", "1"))):
        results, ns = bench_hw(nc, in_maps,
                               iters=int(os.environ.get("GNN_ITERS", "32")))
        LAST_EXEC_NS = ns
    else:
        res = run_bass_kernel_spmd(nc, in_maps, core_ids=list(range(NCORES)))
        results = res.results
        LAST_EXEC_NS = res.exec_time_ns
    NSHARD = cfg.NSHARD
    y_perm = np.concatenate(
        [results[k]["y"].reshape(NSHARD // 512, P, 4, P)
         .transpose(0, 2, 1, 3).reshape(NSHARD, P)
         for k in range(NCORES)], axis=0)
    y = np.empty_like(y_perm)
    y[p2o] = y_perm
    return y.astype(np.float32)


LAST_EXEC_NS = None


# revision 9
# speedup vs baseline: 67.3294x; 1.1741x over previous
"""Trainium2 Bass kernel for the ExplicitV2C GNN layer (GATv2 message passing).

Strategy (8-core SPMD, no collectives):
  * Host: permute nodes into 512 degree-balanced bins of 128 nodes; group
    edges by destination bin; pad each bin to S subtiles of 128 edges.
    Each core owns 64 bins (8192 dst nodes) and all edges targeting them.
  * Device per core:
      Phase 1 (replicated): LLR fusion (Linear+LN+ReLU+mask) over ALL nodes;
        writes the full bf16 x_w table to core-local DRAM (gather source).
      Phase 2 (edges, sharded): batched indirect-DMA gathers of x_w rows
        (2560 rows per DMA op, including each window's own dst nodes), GATv2
        scores with bf16 matmuls, leaky_relu as 0.2*z + 0.8*relu(z),
        segment softmax + weighted aggregation via one-hot matmuls in PSUM.
      Phase 3 (nodes, sharded): degree gate + final LayerNorm; the degree
        embedding term is fetched with a single dma_gather op.
  * Host: reorder the output shards, undo the node permutation.
"""

import os
import sys

sys.path.insert(0, "/opt/trn_rl_repo")

import numpy as np
import ml_dtypes

import concourse.bass as bass
import concourse.bacc as bacc
import concourse.mybir as mybir
import concourse.tile as tile
from concourse.bass import IndirectOffsetOnAxis
from concourse.bass_utils import run_bass_kernel_spmd

F32 = mybir.dt.float32
BF16 = mybir.dt.bfloat16
I32 = mybir.dt.int32
I16 = mybir.dt.int16
AX = mybir.AxisListType
OP = mybir.AluOpType
AF = mybir.ActivationFunctionType

P = 128
NCORES = 8
LN_EPS = 1e-5
SM_EPS = 1e-16
BF = ml_dtypes.bfloat16


class Cfg:
    def __init__(self, N=65536, E=262144, S_SUB=4):
        self.N, self.E, self.S_SUB = N, E, S_SUB
        self.BINS = N // P                       # node bins total (512)
        self.BPC = self.BINS // NCORES           # windows per core (64)
        self.NSHARD = N // NCORES                # nodes per core (8192)
        self.SLOTS = S_SUB * P                   # edge slots per bin
        self.NSLAB = N // 512                    # phase-1 slabs (128)
        self.CH = 16                             # gather chunks per core
        self.WPC = self.BPC // self.CH           # windows per chunk (4)
        self.CPW = S_SUB + 1                     # gather cols per window


# ----------------------------------------------------------------------------
# Host-side preprocessing
# ----------------------------------------------------------------------------

def _balance_bins(deg_in, N, BINS, target):
    """LPT assignment: nodes by in-degree descending onto the lightest bin
    that still has free slots; every bin gets exactly P nodes."""
    import heapq
    order = np.argsort(-deg_in, kind="stable")
    bin_of = np.empty(N, np.int64)
    slot_of = np.empty(N, np.int64)
    heap = [(0, 0, b) for b in range(BINS)]
    heapq.heapify(heap)
    for n in order:
        while True:
            load, cnt, b = heapq.heappop(heap)
            if cnt < P:
                break
        bin_of[n] = b
        slot_of[n] = cnt
        heapq.heappush(heap, (load + int(deg_in[n]), cnt + 1, b))
    loads = np.bincount(bin_of, weights=deg_in, minlength=BINS).astype(np.int64)
    return bin_of, slot_of, loads


def host_prep(cfg, inputs):
    N, E = cfg.N, cfg.E
    BINS, BPC, NSHARD = cfg.BINS, cfg.BPC, cfg.NSHARD

    x = np.asarray(inputs["x"], np.float32)
    ei = np.asarray(inputs["edge_index"])
    src_o = ei[0].astype(np.int64)
    dst_o = ei[1].astype(np.int64)
    ea = np.asarray(inputs["edge_attr"], np.float32)
    ndeg = np.asarray(inputs["node_degrees"]).astype(np.int64)
    llr = np.asarray(inputs["llr_features"], np.float32).reshape(N)
    vmask = np.asarray(inputs["var_node_mask"]).astype(np.float32).reshape(N)

    deg_in = np.bincount(dst_o, minlength=N).astype(np.int64)
    target = -(-E // BINS)
    bin_of, slot_of, loads = _balance_bins(deg_in, N, BINS, target)
    max_load = int(loads.max())
    S = max(1, -(-max_load // P))
    cfg = Cfg(N, E, S)
    SLOTS = cfg.SLOTS
    CH, WPC, CPW = cfg.CH, cfg.WPC, cfg.CPW

    # permuted node id: node o sits at (bin, slot)
    o2p = bin_of * P + slot_of
    p2o = np.argsort(o2p)          # p2o[pid] = original id

    # x_w DRAM table row of permuted node n: n = slab*512 + t*128 + p is
    # stored at row slab*512 + p*4 + t (matches contiguous slab stores)
    n_ids = np.arange(N)
    n_slab = n_ids // 512
    n_t = (n_ids % 512) // P
    n_p = n_ids % P
    row_of_node = n_slab * 512 + n_p * 4 + n_t

    # --- edge arrays grouped by destination bin ---------------------------
    src_p = o2p[src_o]
    dst_pid = o2p[dst_o]
    ebin = dst_pid >> 7
    eslot = dst_pid & 127

    eorder = np.argsort(ebin, kind="stable")
    ebin_s = ebin[eorder]
    starts = np.zeros(BINS + 1, np.int64)
    np.cumsum(np.bincount(ebin_s, minlength=BINS), out=starts[1:])
    rank = np.arange(E) - starts[ebin_s]
    q = ebin_s * SLOTS + rank                 # position in padded layout

    esrc = np.zeros(BINS * SLOTS, np.int64)   # permuted src node id
    eslot_f = np.full(BINS * SLOTS, float(P), np.float32)   # pad slot = P
    eattr = np.zeros((BINS * SLOTS, 8), np.float32)
    esrc[q] = src_p[eorder]
    eslot_f[q] = eslot[eorder].astype(np.float32)
    eattr[q] = ea[eorder]

    # gather row index per edge slot (into the shuffled x_w table layout).
    # Pad slots point at the last table row (positive int16 after re-basing),
    # and each bin's slots are stably partitioned so that positive-row slots
    # come last: the transpose-gather drops trailing NEGATIVE indices, so the
    # final index of every per-window gather op must be non-negative.
    egrow_f = np.full(BINS * SLOTS, N - 1, np.int64)
    filled = np.zeros(BINS * SLOTS, bool)
    filled[q] = True
    egrow_f[q] = row_of_node[src_p[eorder]]
    eg2 = egrow_f.reshape(BINS, SLOTS)
    es2 = eslot_f.reshape(BINS, SLOTS)
    ea2 = eattr.reshape(BINS, SLOTS, 8)
    order2 = np.argsort(eg2 >= N // 2, axis=1, kind="stable")
    eg2 = np.take_along_axis(eg2, order2, axis=1)
    es2 = np.take_along_axis(es2, order2, axis=1)
    ea2 = np.take_along_axis(ea2, order2[:, :, None], axis=1)
    eslot_f = es2.reshape(-1)
    eattr = ea2.reshape(-1, 8)
    egrow = eg2.reshape(BINS, S, P)                   # [win_glob, j, p]

    # per-core transpose-gather indices: int16 = table_row - N/2 (sign trick
    # extends the addressable range to 65536 rows).  Position i = col*128 + e;
    # the CPW cols of window w are [own nodes, edge subtile 0..S-1].
    NPC = WPC * CPW * P                           # idx positions per chunk
    idx_g = np.zeros((NCORES, CH, P, NPC // 16), np.int16)
    half = N // 2
    for c in range(NCORES):
        for ch in range(CH):
            unw = np.zeros(NPC, np.int64)
            for wdx in range(WPC):
                wg = c * BPC + ch * WPC + wdx     # global bin
                base = wdx * CPW * P
                own_nodes = wg * P + np.arange(P) # permuted ids of own bin
                unw[base:base + P] = row_of_node[own_nodes]
                for j in range(S):
                    unw[base + (1 + j) * P:base + (2 + j) * P] = egrow[wg, j]
            w16 = (unw - half).astype(np.int16).reshape(NPC // 16, 16).T
            idx_g[c, ch] = np.tile(w16, (8, 1))

    eslot_r = eslot_f.reshape(NCORES, BPC * S, P)
    dst_col = eslot_r.transpose(0, 2, 1).copy()               # [c, p, col]
    eattr_r = eattr.reshape(NCORES, BPC * S, P, 8)
    ea_t = eattr_r.transpose(0, 3, 1, 2).reshape(
        NCORES, 8, BPC * S * P).astype(BF)                    # [c, 8, col*p]

    # --- node arrays (full, replicated) -----------------------------------
    xp = x[p2o]                                              # [N, HID]
    x_t_full = np.ascontiguousarray(xp.T.astype(BF))         # [128, N]
    # interleaved rows: [slab, p, t, f], node n = slab*512 + t*128 + p
    xr4 = np.ascontiguousarray(
        xp.reshape(cfg.NSLAB, 4, P, P).transpose(0, 2, 1, 3).astype(BF))
    # llr per node: [p, slab, t]
    llr4 = np.ascontiguousarray(
        llr[p2o].reshape(cfg.NSLAB, 4, P).transpose(2, 0, 1).astype(BF))
    # mask: [p, slab, t]
    m_all = np.ascontiguousarray(
        vmask[p2o].reshape(cfg.NSLAB, 4, P).transpose(2, 0, 1).astype(BF))

    # degree gather indices (int16), wrap order, replicated to 128 parts
    degc = np.clip(ndeg, 0, 99)[p2o].reshape(NCORES, NSHARD).astype(np.int16)
    deg_wrap = degc.reshape(NCORES, NSHARD // 16, 16).transpose(0, 2, 1)
    deg_rep = np.ascontiguousarray(np.tile(deg_wrap, (1, 8, 1)))  # [c,128,S]

    # --- weights -----------------------------------------------------------
    w = {k: np.asarray(v, np.float32) for k, v in inputs.items()
         if k not in ("x", "edge_index", "edge_attr", "node_degrees",
                      "llr_features", "var_node_mask")}
    att = w["att"]                                           # [4,128]

    def bcast_row(v, reps):                                  # [P, reps*128]
        return np.ascontiguousarray(
            np.broadcast_to(np.tile(v, reps)[None, :], (P, reps * P)))

    flags = {
        "gf1": bool(np.allclose(w["g_f"], 1.0)),
        "bef0": bool(np.allclose(w["be_f"], 0.0)),
        "gg1": bool(np.allclose(w["g_g"], 1.0)),
        "beg0": bool(np.allclose(w["be_g"], 0.0)),
        "bg20": bool(np.allclose(w["b_g2"], 0.0)),
        "go1": bool(np.allclose(w["g_o"], 1.0)),
        "bo0": bool(np.allclose(w["b_o"], 0.0)),
    }

    consts = {
        "c_Wfx": w["W_f"][:P].astype(BF),                    # [128,128]
        "c_wfl4": np.ascontiguousarray(np.broadcast_to(
            np.tile(w["W_f"][P], 4)[None, :], (P, 512)).astype(BF)),
        "c_bfc": np.ascontiguousarray(
            w["b_f"].reshape(P, 1).astype(np.float32)),
        "c_eps": np.full((P, 1), LN_EPS, np.float32),
        "c_Wl": w["W_l"].astype(BF),                         # [128,512]
        "c_Wr": w["W_r"].astype(BF),
        "c_We": w["W_e"].astype(BF),                         # [8,512]
        "c_attb": np.ascontiguousarray(
            np.broadcast_to(att.reshape(1, 512), (P, 512)).astype(BF)),
        "c_iota": np.ascontiguousarray(
            np.broadcast_to(np.arange(P, dtype=np.float32)[None, :], (P, P))),
        "c_ident": np.eye(P, dtype=BF),
        "c_Wg1x": w["W_g1"][:P].astype(BF),                  # [128,128]
        "c_Wg2": w["W_g2"].astype(BF),
        "c_Td": (w["deg_emb"] @ w["W_g1"][P:P + 16]
                 + w["b_g1"][None, :]).astype(np.float32),   # [100,128]
    }
    if not flags["gf1"]:
        consts["c_gf4"] = bcast_row(w["g_f"], 4).astype(BF)
    if not flags["bef0"]:
        consts["c_bef4"] = bcast_row(w["be_f"], 4).astype(BF)
    if not flags["gg1"]:
        consts["c_gg4"] = bcast_row(w["g_g"], 4).astype(BF)
    if not flags["beg0"]:
        consts["c_beg4"] = bcast_row(w["be_g"], 4).astype(BF)
    if not flags["bg20"]:
        consts["c_bg24"] = bcast_row(w["b_g2"], 4).astype(np.float32)
    if not flags["go1"]:
        consts["c_go4"] = bcast_row(w["g_o"], 4).astype(np.float32)
    if not flags["bo0"]:
        consts["c_bo4"] = bcast_row(w["b_o"], 4).astype(np.float32)
    consts = {k: np.ascontiguousarray(v) for k, v in consts.items()}

    in_maps = []
    for k in range(NCORES):
        m = {
            "x_t": x_t_full, "xr4": xr4, "llr4": llr4, "m_all": m_all,
            "idx_g": np.ascontiguousarray(idx_g[k]),
            "dst_col": np.ascontiguousarray(dst_col[k]),
            "ea_t": np.ascontiguousarray(ea_t[k]),
            "deg_w": deg_rep[k],
        }
        m.update(consts)
        in_maps.append(m)
    return cfg, in_maps, p2o, flags


# ----------------------------------------------------------------------------
# Device kernel
# ----------------------------------------------------------------------------

def build_kernel(cfg, flags):
    PH = int(os.environ.get("GNN_PH", "3"))
    N, S = cfg.N, cfg.S_SUB
    BPC, NSHARD = cfg.BPC, cfg.NSHARD
    NSLAB, CH, WPC, CPW = cfg.NSLAB, cfg.CH, cfg.WPC, cfg.CPW
    SH_SLAB = NSHARD // 512                      # output slabs per core (16)
    COLS = BPC * S                               # edge subtile columns (256)

    nc = bacc.Bacc("TRN2", target_bir_lowering=False, debug=False,
                   num_devices=NCORES)

    # ---- I/O ----
    d_xt = nc.dram_tensor("x_t", [P, N], BF16, kind="ExternalInput")
    d_xr4 = nc.dram_tensor("xr4", [NSLAB, P, 4, P], BF16, kind="ExternalInput")
    d_llr = nc.dram_tensor("llr4", [P, NSLAB, 4], BF16, kind="ExternalInput")
    d_m = nc.dram_tensor("m_all", [P, NSLAB, 4], BF16, kind="ExternalInput")
    d_idx = nc.dram_tensor("idx_g", [CH, P, WPC * CPW * 8], I16,
                           kind="ExternalInput")
    d_dst = nc.dram_tensor("dst_col", [P, COLS], F32, kind="ExternalInput")
    d_eat = nc.dram_tensor("ea_t", [8, COLS * P], BF16, kind="ExternalInput")
    d_deg = nc.dram_tensor("deg_w", [P, NSHARD // 16], I16,
                           kind="ExternalInput")
    d_out = nc.dram_tensor("y", [SH_SLAB, P, 4, P], F32, kind="ExternalOutput")

    cshape = {
        "c_Wfx": ([P, P], BF16), "c_wfl4": ([P, 512], BF16),
        "c_bfc": ([P, 1], F32), "c_eps": ([P, 1], F32),
        "c_Wl": ([P, 512], BF16), "c_Wr": ([P, 512], BF16),
        "c_We": ([8, 512], BF16), "c_attb": ([P, 512], BF16),
        "c_iota": ([P, P], F32), "c_ident": ([P, P], BF16),
        "c_Wg1x": ([P, P], BF16), "c_Wg2": ([P, P], BF16),
        "c_Td": ([100, P], F32),
    }
    for nm, fl, dt in (("c_gf4", "gf1", BF16), ("c_bef4", "bef0", BF16),
                       ("c_gg4", "gg1", BF16), ("c_beg4", "beg0", BF16),
                       ("c_bg24", "bg20", F32), ("c_go4", "go1", F32),
                       ("c_bo4", "bo0", F32)):
        if not flags[fl]:
            cshape[nm] = ([P, 512], dt)
    d_c = {k: nc.dram_tensor(k, sh, dt, kind="ExternalInput")
           for k, (sh, dt) in cshape.items()}

    # gather table with a shadow copy of the first half appended: the
    # transpose-gather reads rows [0,N) via int16 idx relative to base N/2,
    # but its declared AP is rows [N/2, 2N) -- the shadow writes make every
    # phase-1 store overlap that range so the dep tracker orders them.
    d_xw = nc.dram_tensor("xw_tab", [2 * N, P], BF16)
    d_td = nc.dram_tensor("td_tab", [100, P], F32)

    with tile.TileContext(nc) as tc:
        with (
            tc.tile_pool(name="const", bufs=1) as cpool,
            tc.tile_pool(name="resid", bufs=1) as rpool,
        ):
            C = {}
            for k, (sh, dt) in cshape.items():
                C[k] = cpool.tile(sh, dt, tag=k, name=f"const_{k}")
                nc.sync.dma_start(out=C[k][:], in_=d_c[k].ap())
            nc.sync.dma_start(out=d_td.ap(), in_=C["c_Td"][:])

            llr_s = rpool.tile([P, NSLAB, 4], BF16, tag="llr")
            nc.sync.dma_start(out=llr_s[:], in_=d_llr.ap())
            m_s = rpool.tile([P, NSLAB, 4], BF16, tag="m")
            nc.sync.dma_start(out=m_s[:], in_=d_m.ap())
            dst_t = rpool.tile([P, COLS], F32, tag="dst")
            nc.sync.dma_start(out=dst_t[:], in_=d_dst.ap())
            eat_t = rpool.tile([8, COLS * P], BF16, tag="eat")
            nc.sync.dma_start(out=eat_t[:], in_=d_eat.ap())
            deg_t = rpool.tile([P, NSHARD // 16], I16, tag="deg")
            nc.sync.dma_start(out=deg_t[:], in_=d_deg.ap())

            v2c_nm = rpool.tile([P, BPC, P], BF16, tag="v2c")  # [p, w, f]
            dterm = rpool.tile([P, BPC, P], F32, tag="dterm")

            # degree-embedding term for all own nodes (1024-idx chunks --
            # a single instruction's descriptors must fit the SWDGE ring)
            for g in range(NSHARD // 1024):
                nc.gpsimd.dma_gather(
                    out_ap=dterm[:, g * 8:(g + 1) * 8, :], in_ap=d_td.ap(),
                    idxs_ap=deg_t[:, g * 64:(g + 1) * 64],
                    num_idxs=1024, num_idxs_reg=1024, elem_size=P,
                    transpose=False)

            # ================= Phase 1: LLR fusion (replicated) =============
            with (
                tc.tile_pool(name="p1mm", bufs=2, space="PSUM") as pp1,
                tc.tile_pool(name="p1tr", bufs=2, space="PSUM") as pp1t,
                tc.tile_pool(name="p1in", bufs=3) as sb1i,
                tc.tile_pool(name="p1wk", bufs=2) as sb1,
            ):
                for s in range(NSLAB):
                    ns = slice(s * 512, (s + 1) * 512)
                    xt_sl = sb1i.tile([P, 512], BF16, tag="xt")
                    nc.sync.dma_start(out=xt_sl[:], in_=d_xt.ap()[:, ns])
                    py = pp1.tile([P, 512], F32, tag="y")
                    nc.tensor.matmul(py[:], C["c_Wfx"][:], xt_sl[:],
                                     start=True, stop=True)
                    ytT = sb1.tile([P, 512], BF16, tag="ytT")
                    nc.scalar.activation(ytT[:], py[:], AF.Identity,
                                         bias=C["c_bfc"][:])
                    ptq = pp1t.tile([P, 512], BF16, tag="tr")
                    for t in range(4):
                        qs = slice(t * P, (t + 1) * P)
                        nc.tensor.transpose(ptq[:, qs], ytT[:, qs],
                                            C["c_ident"][:])
                    wl4 = sb1.tile([P, 4, P], BF16, tag="wl4")
                    nc.vector.tensor_tensor(
                        out=wl4[:],
                        in0=C["c_wfl4"][:].rearrange("p (t f) -> p t f", t=4),
                        in1=llr_s[:, s, :].rearrange("p (t o) -> p t o", o=1)
                            .to_broadcast([P, 4, P]), op=OP.mult)
                    yr = sb1.tile([P, 4, P], BF16, tag="yr")
                    nc.vector.tensor_tensor(
                        out=yr[:], in0=ptq[:].rearrange("p (t f) -> p t f", t=4),
                        in1=wl4[:], op=OP.add)
                    bst = sb1.tile([P, 4, 6], F32, tag="bst")
                    mv = sb1.tile([P, 4, 2], F32, tag="mv")
                    for t in range(4):
                        nc.vector.bn_stats(bst[:, t, :], yr[:, t, :])
                        nc.vector.bn_aggr(mv[:, t, :], bst[:, t, :])
                    sd4 = sb1.tile([P, 4], F32, tag="sd4")
                    nc.scalar.activation(sd4[:], mv[:, :, 1], AF.Sqrt,
                                         bias=C["c_eps"][:])
                    iv4 = sb1.tile([P, 4], F32, tag="iv4")
                    nc.vector.reciprocal(iv4[:], sd4[:])
                    nm4 = sb1.tile([P, 4], F32, tag="nm4")
                    nc.vector.scalar_tensor_tensor(
                        out=nm4[:], in0=mv[:, :, 0], scalar=-1.0, in1=iv4[:],
                        op0=OP.mult, op1=OP.mult)
                    t1 = sb1.tile([P, 4, P], BF16, tag="t1")
                    nc.vector.tensor_tensor(
                        out=t1[:], in0=yr[:],
                        in1=iv4[:].rearrange("p (t o) -> p t o", o=1)
                            .to_broadcast([P, 4, P]), op=OP.mult)
                    t2 = sb1.tile([P, 4, P], BF16, tag="t2")
                    nc.vector.tensor_tensor(
                        out=t2[:], in0=t1[:],
                        in1=nm4[:].rearrange("p (t o) -> p t o", o=1)
                            .to_broadcast([P, 4, P]), op=OP.add)
                    zz = t2
                    if not flags["gf1"]:
                        zg = sb1.tile([P, 4, P], BF16, tag="zg")
                        nc.vector.tensor_tensor(
                            out=zg[:], in0=zz[:],
                            in1=C["c_gf4"][:].rearrange(
                                "p (t f) -> p t f", t=4), op=OP.mult)
                        zz = zg
                    if not flags["bef0"]:
                        zb = sb1.tile([P, 4, P], BF16, tag="zb")
                        nc.vector.tensor_tensor(
                            out=zb[:], in0=zz[:],
                            in1=C["c_bef4"][:].rearrange(
                                "p (t f) -> p t f", t=4), op=OP.add)
                        zz = zb
                    fu = sb1.tile([P, 4, P], BF16, tag="fu")
                    nc.scalar.activation(fu[:], zz[:], AF.Relu)
                    xr_sl = sb1i.tile([P, 4, P], BF16, tag="xr")
                    nc.scalar.dma_start(out=xr_sl[:], in_=d_xr4.ap()[s])
                    d1 = sb1.tile([P, 4, P], BF16, tag="d1")
                    nc.vector.tensor_tensor(out=d1[:], in0=fu[:], in1=xr_sl[:],
                                            op=OP.subtract)
                    dm = sb1.tile([P, 4, P], BF16, tag="dm")
                    nc.vector.tensor_tensor(
                        out=dm[:], in0=d1[:],
                        in1=m_s[:, s, :].rearrange("p (t o) -> p t o", o=1)
                            .to_broadcast([P, 4, P]), op=OP.mult)
                    xw_sl = sb1.tile([P, 4, P], BF16, tag="xw")
                    nc.vector.tensor_tensor(out=xw_sl[:], in0=dm[:],
                                            in1=xr_sl[:], op=OP.add)
                    nc.sync.dma_start(
                        out=d_xw.ap()[ns, :].rearrange(
                            "(pp t) f -> pp t f", t=4),
                        in_=xw_sl[:])
                    if s < NSLAB // 2:
                        sh = slice(N + s * 512, N + (s + 1) * 512)
                        nc.scalar.dma_start(
                            out=d_xw.ap()[sh, :].rearrange(
                                "(pp t) f -> pp t f", t=4),
                            in_=xw_sl[:])
                    if PH == 1 and s < SH_SLAB:
                        dbg = sb1.tile([P, 4, P], F32, tag="dbg")
                        nc.vector.tensor_copy(out=dbg[:], in_=xw_sl[:])
                        nc.scalar.dma_start(out=d_out.ap()[s], in_=dbg[:])

            # ================= Phase 2: edges ===============================
            with (
                tc.tile_pool(name="pz", bufs=2, space="PSUM") as ppz,
                tc.tile_pool(name="pxl", bufs=2, space="PSUM") as ppxl,
                tc.tile_pool(name="po4", bufs=1, space="PSUM") as ppo4,
                tc.tile_pool(name="psm", bufs=1, space="PSUM") as ppsm,
                tc.tile_pool(name="ptr", bufs=2, space="PSUM") as pptr,
                tc.tile_pool(name="e_in", bufs=2) as ein,
                tc.tile_pool(name="e_wk", bufs=3) as ewk,
                tc.tile_pool(name="e_w2", bufs=2) as ewk2,
            ):
                NPC = WPC * CPW * P
                NPW = CPW * P                      # idx positions per window
                for ch in range(CH if PH >= 2 else 0):
                    idx_t = ein.tile([P, NPC // 16], I16, tag="idx")
                    nc.sync.dma_start(out=idx_t[:], in_=d_idx.ap()[ch])
                    # feature-major gather: column i holds x_w of idx i
                    xg_t = ein.tile([P, 1, NPC], BF16, tag="xg")
                    for wdx in range(WPC):
                        nc.gpsimd.dma_gather(
                            out_ap=xg_t[:, :, wdx * NPW:(wdx + 1) * NPW],
                            in_ap=d_xw.ap()[N // 2:2 * N, :],
                            idxs_ap=idx_t[:, wdx * (NPW // 16):
                                          (wdx + 1) * (NPW // 16)],
                            num_idxs=NPW, num_idxs_reg=NPW,
                            elem_size=P, transpose=True)

                    for wdx in range(WPC):
                        win = ch * WPC + wdx
                        base = wdx * CPW * P
                        # xr tile for this window's own (dst) nodes
                        xwT = xg_t[:, 0, base:base + P]
                        pxr = ppz.tile([P, 512], F32, tag="z")
                        nc.tensor.matmul(pxr[:], xwT, C["c_Wr"][:],
                                         start=True, stop=True)
                        xr_sb = ewk2.tile([P, 512], BF16, tag="xr")
                        nc.vector.tensor_copy(out=xr_sb[:], in_=pxr[:])

                        pden = ppsm.tile([P, 4], F32, tag="sm")
                        po4 = ppo4.tile([P, 512], F32, tag="o4")

                        for j in range(S):
                            st = win * S + j
                            # one-hot S [e, d] and its transpose
                            S_sb = ewk.tile([P, P], BF16, tag="S")
                            nc.vector.tensor_tensor(
                                out=S_sb[:],
                                in0=dst_t[:, st:st + 1].to_broadcast([P, P]),
                                in1=C["c_iota"][:], op=OP.is_equal)
                            pts = pptr.tile([P, P], BF16, tag="tr")
                            nc.tensor.transpose(pts[:], S_sb[:],
                                                C["c_ident"][:])
                            st_sb = ewk.tile([P, P], BF16, tag="st")
                            nc.scalar.activation(st_sb[:], pts[:], AF.Identity)
                            # gathered x_w[src] columns (feature-major)
                            xgT = xg_t[:, 0, base + (1 + j) * P:
                                       base + (2 + j) * P]

                            ea_sl = eat_t[:, st * P:(st + 1) * P]
                            pz = ppz.tile([P, 512], F32, tag="z")
                            pxl = ppxl.tile([P, 512], F32, tag="xl")
                            nc.tensor.matmul(pz[:], xgT, C["c_Wl"][:],
                                             start=True, stop=False)
                            nc.tensor.matmul(pxl[:], xgT, C["c_Wl"][:],
                                             start=True, stop=True)
                            nc.tensor.matmul(pz[:], st_sb[:], xr_sb[:],
                                             start=False, stop=False)
                            nc.tensor.matmul(pz[:], ea_sl, C["c_We"][:],
                                             start=False, stop=True)

                            # leaky = 0.2*z + 0.8*relu(z)
                            r_sb = ewk.tile([P, 512], BF16, tag="r")
                            nc.scalar.activation(r_sb[:], pz[:], AF.Relu,
                                                 scale=0.8)
                            lk = ewk.tile([P, 512], BF16, tag="lk")
                            nc.vector.scalar_tensor_tensor(
                                out=lk[:], in0=pz[:], scalar=0.2, in1=r_sb[:],
                                op0=OP.mult, op1=OP.add)
                            # alpha[e,h] = sum_c lk*att
                            zat = ewk.tile([P, 512], BF16, tag="zat")
                            nc.vector.tensor_tensor(out=zat[:], in0=lk[:],
                                                    in1=C["c_attb"][:],
                                                    op=OP.mult)
                            alpha = ewk.tile([P, 4], F32, tag="alpha")
                            nc.vector.reduce_sum(
                                out=alpha[:],
                                in_=zat[:].rearrange("p (h c) -> p h c", h=4),
                                axis=AX.X)
                            au = ewk.tile([P, 4], BF16, tag="au")
                            nc.scalar.activation(au[:], alpha[:], AF.Exp)
                            nc.tensor.matmul(pden[:], S_sb[:], au[:],
                                             start=(j == 0), stop=(j == S - 1))
                            # xl scaled by per-edge attention (per head)
                            xla = ewk.tile([P, 4, P], BF16, tag="xla")
                            nc.vector.tensor_tensor(
                                out=xla[:],
                                in0=pxl[:].rearrange("p (h f) -> p h f", h=4),
                                in1=au[:].rearrange("p (h o) -> p h o", o=1)
                                    .to_broadcast([P, 4, P]), op=OP.mult)
                            nc.tensor.matmul(
                                po4[:], S_sb[:],
                                xla[:].rearrange("p h f -> p (h f)"),
                                start=(j == 0), stop=(j == S - 1))
                        # normalize + head mean -> v2c (node-major)
                        dv = ewk.tile([P, 4], F32, tag="dv")
                        nc.vector.tensor_scalar(out=dv[:], in0=pden[:],
                                                scalar1=SM_EPS, scalar2=None,
                                                op0=OP.add)
                        iv = ewk.tile([P, 4], F32, tag="iv")
                        nc.vector.reciprocal(iv[:], dv[:])
                        nc.vector.tensor_scalar(out=iv[:], in0=iv[:],
                                                scalar1=0.25, scalar2=None,
                                                op0=OP.mult)
                        vsl = v2c_nm[:, win, :]
                        nc.vector.tensor_scalar(
                            out=vsl, in0=po4[:, 0:P], scalar1=iv[:, 0:1],
                            scalar2=None, op0=OP.mult)
                        for h in range(1, 4):
                            hs = slice(h * P, (h + 1) * P)
                            nc.vector.scalar_tensor_tensor(
                                out=vsl, in0=po4[:, hs], scalar=iv[:, h:h + 1],
                                in1=vsl, op0=OP.mult, op1=OP.add)

            # ================= Phase 3: degree gate + final LN ==============
            with (
                tc.tile_pool(name="p3a", bufs=2, space="PSUM") as pp3,
                tc.tile_pool(name="p3t", bufs=2, space="PSUM") as pp3t,
                tc.tile_pool(name="g_wk", bufs=2) as gwk,
            ):
                for sl in range(SH_SLAB if PH >= 3 else 0):
                    ws = slice(sl * 4, sl * 4 + 4)
                    # h = v2c @ Wg1x + dterm (deg-emb term incl. b_g1)
                    ph = pp3.tile([P, 512], F32, tag="h")
                    for t in range(4):
                        win = sl * 4 + t
                        ptv = pp3t.tile([P, P], BF16, tag="t")
                        nc.tensor.transpose(ptv[:], v2c_nm[:, win, :],
                                            C["c_ident"][:])
                        v2cT = gwk.tile([P, P], BF16, tag="v2cT")
                        nc.scalar.activation(v2cT[:], ptv[:], AF.Identity)
                        nc.tensor.matmul(ph[:, t * P:(t + 1) * P], v2cT[:],
                                         C["c_Wg1x"][:], start=True, stop=True)
                    h_sb = gwk.tile([P, 4, P], BF16, tag="h_sb")
                    nc.vector.tensor_tensor(
                        out=h_sb[:],
                        in0=ph[:].rearrange("p (t f) -> p t f", t=4),
                        in1=dterm[:, ws, :], op=OP.add)
                    bst = gwk.tile([P, 4, 6], F32, tag="bst")
                    mv = gwk.tile([P, 4, 2], F32, tag="mv")
                    for t in range(4):
                        nc.vector.bn_stats(bst[:, t, :], h_sb[:, t, :])
                        nc.vector.bn_aggr(mv[:, t, :], bst[:, t, :])
                    sd4 = gwk.tile([P, 4], F32, tag="sd4")
                    nc.scalar.activation(sd4[:], mv[:, :, 1], AF.Sqrt,
                                         bias=C["c_eps"][:])
                    iv4 = gwk.tile([P, 4], F32, tag="iv4")
                    nc.vector.reciprocal(iv4[:], sd4[:])
                    nm4 = gwk.tile([P, 4], F32, tag="nm4")
                    nc.vector.scalar_tensor_tensor(
                        out=nm4[:], in0=mv[:, :, 0], scalar=-1.0, in1=iv4[:],
                        op0=OP.mult, op1=OP.mult)
                    t1 = gwk.tile([P, 4, P], BF16, tag="t1")
                    nc.vector.tensor_tensor(
                        out=t1[:], in0=h_sb[:],
                        in1=iv4[:].rearrange("p (t o) -> p t o", o=1)
                            .to_broadcast([P, 4, P]), op=OP.mult)
                    t2 = gwk.tile([P, 4, P], BF16, tag="t2")
                    nc.vector.tensor_tensor(
                        out=t2[:], in0=t1[:],
                        in1=nm4[:].rearrange("p (t o) -> p t o", o=1)
                            .to_broadcast([P, 4, P]), op=OP.add)
                    zz = t2
                    if not flags["gg1"]:
                        zg = gwk.tile([P, 4, P], BF16, tag="zg")
                        nc.vector.tensor_tensor(
                            out=zg[:], in0=zz[:],
                            in1=C["c_gg4"][:].rearrange(
                                "p (t f) -> p t f", t=4), op=OP.mult)
                        zz = zg
                    if not flags["beg0"]:
                        zb = gwk.tile([P, 4, P], BF16, tag="zb")
                        nc.vector.tensor_tensor(
                            out=zb[:], in0=zz[:],
                            in1=C["c_beg4"][:].rearrange(
                                "p (t f) -> p t f", t=4), op=OP.add)
                        zz = zb
                    h2 = gwk.tile([P, 4, P], BF16, tag="h2")
                    nc.scalar.activation(h2[:], zz[:], AF.Relu)
                    # gate = sigmoid(h2 @ Wg2 + b_g2)
                    pg = pp3.tile([P, 512], F32, tag="h")
                    for t in range(4):
                        pth = pp3t.tile([P, P], BF16, tag="t")
                        nc.tensor.transpose(pth[:], h2[:, t, :],
                                            C["c_ident"][:])
                        h2T = gwk.tile([P, P], BF16, tag="h2T")
                        nc.scalar.activation(h2T[:], pth[:], AF.Identity)
                        nc.tensor.matmul(pg[:, t * P:(t + 1) * P], h2T[:],
                                         C["c_Wg2"][:], start=True, stop=True)
                    gsrc = pg[:]
                    if not flags["bg20"]:
                        gp = gwk.tile([P, 512], F32, tag="gp")
                        nc.vector.tensor_tensor(out=gp[:], in0=pg[:],
                                                in1=C["c_bg24"][:], op=OP.add)
                        gsrc = gp[:]
                    gate = gwk.tile([P, 4, P], BF16, tag="gate")
                    nc.scalar.activation(
                        gate[:], gsrc.rearrange("p (t f) -> p t f", t=4),
                        AF.Sigmoid)
                    p_sb = gwk.tile([P, 4, P], BF16, tag="p_sb")
                    nc.vector.tensor_tensor(out=p_sb[:], in0=v2c_nm[:, ws, :],
                                            in1=gate[:], op=OP.mult)
                    # final LN -> f32 out
                    fbst = gwk.tile([P, 4, 6], F32, tag="fbst")
                    fmv = gwk.tile([P, 4, 2], F32, tag="fmv")
                    for t in range(4):
                        nc.vector.bn_stats(fbst[:, t, :], p_sb[:, t, :])
                        nc.vector.bn_aggr(fmv[:, t, :], fbst[:, t, :])
                    fsd = gwk.tile([P, 4], F32, tag="fsd")
                    nc.scalar.activation(fsd[:], fmv[:, :, 1], AF.Sqrt,
                                         bias=C["c_eps"][:])
                    fiv = gwk.tile([P, 4], F32, tag="fiv")
                    nc.vector.reciprocal(fiv[:], fsd[:])
                    fnm = gwk.tile([P, 4], F32, tag="fnm")
                    nc.vector.scalar_tensor_tensor(
                        out=fnm[:], in0=fmv[:, :, 0], scalar=-1.0, in1=fiv[:],
                        op0=OP.mult, op1=OP.mult)
                    y1 = gwk.tile([P, 4, P], F32, tag="y1")
                    nc.vector.tensor_tensor(
                        out=y1[:], in0=p_sb[:],
                        in1=fiv[:].rearrange("p (t o) -> p t o", o=1)
                            .to_broadcast([P, 4, P]), op=OP.mult)
                    y2 = gwk.tile([P, 4, P], F32, tag="y2")
                    nc.vector.tensor_tensor(
                        out=y2[:], in0=y1[:],
                        in1=fnm[:].rearrange("p (t o) -> p t o", o=1)
                            .to_broadcast([P, 4, P]), op=OP.add)
                    yy = y2
                    if not flags["go1"]:
                        y3 = gwk.tile([P, 4, P], F32, tag="y3")
                        nc.vector.tensor_tensor(
                            out=y3[:], in0=yy[:],
                            in1=C["c_go4"][:].rearrange(
                                "p (t f) -> p t f", t=4), op=OP.mult)
                        yy = y3
                    if not flags["bo0"]:
                        y4 = gwk.tile([P, 4, P], F32, tag="y4")
                        nc.vector.tensor_tensor(
                            out=y4[:], in0=yy[:],
                            in1=C["c_bo4"][:].rearrange(
                                "p (t f) -> p t f", t=4), op=OP.add)
                        yy = y4
                    nc.scalar.dma_start(out=d_out.ap()[sl], in_=yy[:])
                if PH == 2:
                    for sl in range(SH_SLAB):
                        ws = slice(sl * 4, sl * 4 + 4)
                        dbg = gwk.tile([P, 4, P], F32, tag="dbg2")
                        nc.vector.tensor_copy(out=dbg[:], in_=v2c_nm[:, ws, :])
                        nc.scalar.dma_start(out=d_out.ap()[sl], in_=dbg[:])

    nc.compile()
    return nc


# ----------------------------------------------------------------------------
# Entry point
# ----------------------------------------------------------------------------

_CACHE = {}


def _get_kernel(cfg, flags):
    key = (cfg.N, cfg.E, cfg.S_SUB, tuple(sorted(flags.items())))
    if key not in _CACHE:
        _CACHE[key] = build_kernel(cfg, flags)
    return _CACHE[key]


def bench_hw(nc, in_maps, iters=32):
    """Build the sharded PJRT callable once; time repeated executions.

    Output buffers are zero-filled ON DEVICE each iteration (no host
    upload in the timed loop).
    """
    import time
    import jax
    from jax.sharding import Mesh, PartitionSpec, NamedSharding
    from jax.experimental.shard_map import shard_map
    import concourse.mybir as mb
    from concourse import bass2jax as b2j

    b2j.install_neuronx_cc_hook()
    n_cores = len(in_maps)
    partition_name = (nc.partition_id_tensor.name
                      if nc.partition_id_tensor else None)
    in_names, out_names, out_avals, zero_outs = [], [], [], []
    for alloc in nc.m.functions[0].allocations:
        if not isinstance(alloc, mb.MemoryLocationSet):
            continue
        name = alloc.memorylocations[0].name
        if alloc.kind == "ExternalInput":
            if name != partition_name:
                in_names.append(name)
        elif alloc.kind == "ExternalOutput":
            out_names.append(name)
            shape = tuple(alloc.tensor_shape)
            dtype = mb.dt.np(alloc.dtype)
            out_avals.append(jax.core.ShapedArray(shape, dtype))
            zero_outs.append(np.zeros(shape, dtype))
    n_params = len(in_names)
    n_outs = len(out_avals)
    in_names.extend(out_names)
    if partition_name is not None:
        in_names.append(partition_name)
    donate = tuple(range(n_params, n_params + n_outs))

    chain = max(1, int(os.environ.get("GNN_CHAIN", "1")))

    def _body(*args):
        ins = list(args[:n_params])
        outs = list(args[n_params:])
        # chain several executions per dispatch; the (fully overwritten)
        # output operands thread through so XLA cannot CSE the calls
        for _ in range(chain):
            operands = ins + outs
            if partition_name is not None:
                operands.append(b2j.partition_id_tensor())
            outs = list(b2j._bass_exec_p.bind(
                *operands,
                out_avals=tuple(out_avals), in_names=tuple(in_names),
                out_names=tuple(out_names), lowering_input_output_aliases=(),
                sim_require_finite=True, sim_require_nnan=True, nc=nc))
        return tuple(outs)

    devices = jax.devices()[:n_cores]
    mesh = Mesh(np.asarray(devices), ("core",))
    sharded = jax.jit(
        shard_map(_body, mesh=mesh,
                  in_specs=(PartitionSpec("core"),) * (n_params + n_outs),
                  out_specs=(PartitionSpec("core"),) * n_outs,
                  check_rep=False),
        donate_argnums=donate, keep_unused=True)

    concat_in = [
        np.concatenate([np.asarray(in_maps[c][in_names[i]])
                        for c in range(n_cores)], axis=0)
        for i in range(n_params)]
    in_shardings = [NamedSharding(mesh, PartitionSpec("core"))] * n_params
    in_bufs = [jax.device_put(a, s) for a, s in zip(concat_in, in_shardings)]

    import jax.numpy as jnp
    zero_sharding = tuple(
        NamedSharding(mesh, PartitionSpec("core")) for _ in range(n_outs))
    zeros_jit = jax.jit(
        lambda: tuple(
            jnp.zeros((n_cores * z.shape[0], *z.shape[1:]), z.dtype)
            for z in zero_outs),
        out_shardings=zero_sharding)

    def fresh_zeros():
        return list(zeros_jit())

    out_arrs = sharded(*in_bufs, *fresh_zeros())
    jax.block_until_ready(out_arrs)
    results = [
        {name: np.asarray(out_arrs[i]).reshape(n_cores, *out_avals[i].shape)[c]
         for i, name in enumerate(out_names)}
        for c in range(n_cores)]

    # pre-create all zero sets so the timed loop only dispatches the NEFF
    zsets = [fresh_zeros() for _ in range(iters)]
    jax.block_until_ready(zsets)

    t0 = time.perf_counter()
    outs = [sharded(*in_bufs, *z) for z in zsets]
    jax.block_until_ready(outs)
    dt = (time.perf_counter() - t0) / (iters * chain)
    return results, dt * 1e9


def kernel(**inputs):
    global LAST_EXEC_NS
    N, E = 65536, 262144
    cfg = Cfg(N, E)
    cfg, in_maps, p2o, flags = host_prep(cfg, inputs)
    nc = _get_kernel(cfg, flags)
    if bool(int(os.environ.get("GNN_BENCH", "1"))):
        results, ns = bench_hw(nc, in_maps,
                               iters=int(os.environ.get("GNN_ITERS", "32")))
        LAST_EXEC_NS = ns
    else:
        res = run_bass_kernel_spmd(nc, in_maps, core_ids=list(range(NCORES)))
        results = res.results
        LAST_EXEC_NS = res.exec_time_ns
    NSHARD = cfg.NSHARD
    y_perm = np.concatenate(
        [results[k]["y"].reshape(NSHARD // 512, P, 4, P)
         .transpose(0, 2, 1, 3).reshape(NSHARD, P)
         for k in range(NCORES)], axis=0)
    y = np.empty_like(y_perm)
    y[p2o] = y_perm
    return y.astype(np.float32)


LAST_EXEC_NS = None
